# revision 23
# baseline (speedup 1.0000x reference)
"""MinVQVAE1D forward pass on 8 Trainium2 NeuronCores.

Data-parallel: batch N=16384 sharded 2048 rows/core; codebook + MLP weights
replicated. All matmuls run in float32r (fp32 storage, TF32-like PE mode at
bf16 speed). The VQ argmin is computed as argmax of (z_e . e_k - ||e_k||^2/2)
where the -c/2 term is folded into the PE accumulation as two K=1 ones-row
matmuls (hi+lo split so the c term keeps ~fp32 accuracy). Index extraction
uses the DVE max/max_index top-8 instructions; z_q rows come back via an
indirect-DMA gather from HBM. Loss partial sums are produced on-device and
finished on host.

Self-contained: hardcodes all shapes from the problem spec.
"""
import sys

sys.path.insert(0, "/opt/trn_rl_repo")

import numpy as np

import concourse.bass as bass
import concourse.mybir as mybir
import concourse.tile as tile
from concourse import bacc
from concourse.bass import IndirectOffsetOnAxis
from concourse.bass_utils import run_bass_kernel_spmd

# problem shapes
N, D, H, L, K = 16384, 1024, 1024, 256, 4096
NCORES = 8
NS = N // NCORES          # rows per core
P = 128
RBLK = 256                # supertile row block (moving free dim; >=256 keeps f32r at 1 cyc/row)
NST = NS // RBLK          # supertiles per core
NTILE = NS // P           # 128-row tiles per core (VQ phase)
KC = 512                  # distance k-chunk (one PSUM bank)
NKC = K // KC

F32 = mybir.dt.float32
F32R = mybir.dt.float32r
AF = mybir.ActivationFunctionType
ALU = mybir.AluOpType

_cache = {}


def _build(repeat=1):
    nc = bacc.Bacc(None, target_bir_lowering=False)

    # ---- DRAM I/O ----
    # xT pre-tiled on host: [NST, 128, D//128, RBLK]; one contiguous-per-
    # partition DMA per supertile (128 descriptors x 8KB).
    xT_d = nc.dram_tensor("xT", [NST, P, D // P, RBLK], F32, kind="ExternalInput")
    eT_d = nc.dram_tensor("eT", [P, L // P, K], F32R, kind="ExternalInput")
    ep_d = nc.dram_tensor("ep", [K, L], F32, kind="ExternalInput")
    ew1_d = nc.dram_tensor("ew1", [P, D // P, H], F32R, kind="ExternalInput")
    ew2_d = nc.dram_tensor("ew2", [P, H // P, H], F32R, kind="ExternalInput")
    ew3_d = nc.dram_tensor("ew3", [P, H // P, L], F32R, kind="ExternalInput")
    dw1_d = nc.dram_tensor("dw1", [P, L // P, H], F32R, kind="ExternalInput")
    dw2_d = nc.dram_tensor("dw2", [P, H // P, H], F32R, kind="ExternalInput")
    dw3_d = nc.dram_tensor("dw3", [P, H // P, D], F32R, kind="ExternalInput")
    # biases pre-shaped [128, nchunks] on host
    eb1_d = nc.dram_tensor("eb1", [P, H // P], F32, kind="ExternalInput")
    eb2_d = nc.dram_tensor("eb2", [P, H // P], F32, kind="ExternalInput")
    eb3_d = nc.dram_tensor("eb3", [P, L // P], F32, kind="ExternalInput")
    db1_d = nc.dram_tensor("db1", [P, H // P], F32, kind="ExternalInput")
    db2_d = nc.dram_tensor("db2", [P, H // P], F32, kind="ExternalInput")
    db3_d = nc.dram_tensor("db3", [P, D // P], F32, kind="ExternalInput")
    iota_d = nc.dram_tensor("iota16", [P, K], mybir.dt.int16, kind="ExternalInput")
    ident_d = nc.dram_tensor("ident", [P, P], F32, kind="ExternalInput")
    ones1_d = nc.dram_tensor("ones1", [1, P], F32R, kind="ExternalInput")
    ones128_d = nc.dram_tensor("ones128", [P, 1], F32R, kind="ExternalInput")

    xpT_d = nc.dram_tensor("xpredT", [NST, P, D // P, RBLK], F32, kind="ExternalOutput")
    oh_d = nc.dram_tensor("onehot", [NS, K], mybir.dt.int32, kind="ExternalOutput")
    lossp_d = nc.dram_tensor("lossp", [P, 2], F32, kind="ExternalOutput")

    xT_r = xT_d.ap()
    xpT_r = xpT_d.ap()
    ew1_r = ew1_d.ap()
    ew2_r = ew2_d.ap()
    ew3_r = ew3_d.ap()
    dw1_r = dw1_d.ap()
    dw2_r = dw2_d.ap()
    dw3_r = dw3_d.ap()
    eT_r = eT_d.ap()
    oh_r = oh_d.ap()

    with tile.TileContext(nc) as tc:
        import contextlib

        stack = contextlib.ExitStack()
        with stack:
            persist = stack.enter_context(tc.tile_pool(name="persist", bufs=1))
            ps_mm = stack.enter_context(tc.tile_pool(name="ps_mm", bufs=3, space="PSUM"))
            ps_big = stack.enter_context(tc.tile_pool(name="ps_big", bufs=3, space="PSUM"))
            ps_tp = stack.enter_context(tc.tile_pool(name="ps_tp", bufs=2, space="PSUM"))

            # ---- persistent small tensors ----
            dw1_t = persist.tile([P, L // P, H], F32R, tag="dw1")
            nc.sync.dma_start(dw1_t[:], dw1_r[:])
            eb1_t = persist.tile([P, H // P], F32, tag="eb1")
            eb2_t = persist.tile([P, H // P], F32, tag="eb2")
            eb3_t = persist.tile([P, L // P], F32, tag="eb3")
            db1_t = persist.tile([P, H // P], F32, tag="db1")
            db2_t = persist.tile([P, H // P], F32, tag="db2")
            db3_t = persist.tile([P, D // P], F32, tag="db3")
            for t, d in [(eb1_t, eb1_d), (eb2_t, eb2_d), (eb3_t, eb3_d),
                         (db1_t, db1_d), (db2_t, db2_d), (db3_t, db3_d)]:
                nc.sync.dma_start(t[:], d.ap())
            ident_t = persist.tile([P, P], F32, tag="ident")
            nc.sync.dma_start(ident_t[:], ident_d.ap())
            ones1_t = persist.tile([1, P], F32R, tag="ones1")
            nc.sync.dma_start(ones1_t[:], ones1_d.ap())
            ones128_t = persist.tile([P, 1], F32R, tag="ones128")
            nc.sync.dma_start(ones128_t[:], ones128_d.ap())
            mch_hi = persist.tile([1, K], F32R, tag="mch_hi")
            mch_lo = persist.tile([1, K], F32R, tag="mch_lo")
            zeT = persist.tile([P, L // P, NS], F32R, tag="zeT")    # 2 MB
            zqT = persist.tile([P, L // P, NS], F32R, tag="zqT")    # 2 MB
            s1buf = persist.tile([P, NST], F32, tag="s1buf")
            s2buf = persist.tile([P, NTILE], F32, tag="s2buf")

            if repeat > 1:
                stack.enter_context(tc.For_i(0, repeat, 1))

            # ================= P1: encoder =================
            with tc.tile_pool(name="encw", bufs=1) as encw, \
                 tc.tile_pool(name="encwork", bufs=2) as work:
                ew1_t = encw.tile([P, D // P, H], F32R, tag="ew1")
                ew2_t = encw.tile([P, H // P, H], F32R, tag="ew2")
                ew3_t = encw.tile([P, H // P, L], F32R, tag="ew3")
                nc.sync.dma_start(ew1_t[:], ew1_r[:])
                nc.sync.dma_start(ew2_t[:], ew2_r[:])
                nc.sync.dma_start(ew3_t[:], ew3_r[:])

                for st in range(NST):
                    rs = st * RBLK
                    xt = work.tile([P, D // P, RBLK], F32R, tag="xt")
                    nc.sync.dma_start(xt[:], xT_r[st].bitcast(F32R))
                    h1 = work.tile([P, H // P, RBLK], F32R, tag="h1")
                    for f in range(H // P):
                        pt = ps_mm.tile([P, RBLK], F32, tag="ps_enc")
                        for d_ in range(D // P):
                            nc.tensor.matmul(
                                pt[:], ew1_t[:, d_, f * P:(f + 1) * P], xt[:, d_, :],
                                start=(d_ == 0), stop=(d_ == D // P - 1),
                            )
                        nc.scalar.activation(h1[:, f, :], pt[:], AF.Gelu,
                                             bias=eb1_t[:, f:f + 1])
                    h2 = work.tile([P, H // P, RBLK], F32R, tag="h2")
                    for f in range(H // P):
                        pt = ps_mm.tile([P, RBLK], F32, tag="ps_enc")
                        for d_ in range(H // P):
                            nc.tensor.matmul(
                                pt[:], ew2_t[:, d_, f * P:(f + 1) * P], h1[:, d_, :],
                                start=(d_ == 0), stop=(d_ == H // P - 1),
                            )
                        nc.scalar.activation(h2[:, f, :], pt[:], AF.Gelu,
                                             bias=eb2_t[:, f:f + 1])
                    for f in range(L // P):
                        pt = ps_mm.tile([P, RBLK], F32, tag="ps_enc")
                        for d_ in range(H // P):
                            nc.tensor.matmul(
                                pt[:], ew3_t[:, d_, f * P:(f + 1) * P], h2[:, d_, :],
                                start=(d_ == 0), stop=(d_ == H // P - 1),
                            )
                        nc.scalar.activation(zeT[:, f, rs:rs + RBLK], pt[:],
                                             AF.Identity, bias=eb3_t[:, f:f + 1])

            # ================= P2: VQ =================
            with tc.tile_pool(name="vq", bufs=1) as vq, \
                 tc.tile_pool(name="vqwork", bufs=2) as vwork, \
                 tc.tile_pool(name="ohpool", bufs=2) as ohp, \
                 tc.tile_pool(name="sqpool", bufs=2) as sqp:
                et_t = vq.tile([P, L // P, K], F32R, tag="et")
                nc.sync.dma_start(et_t[:], eT_r[:])
                iota_t = vq.tile([P, K], mybir.dt.int16, tag="iota")
                nc.sync.dma_start(iota_t[:], iota_d.ap())
                mch_f = vq.tile([1, K], F32, tag="mch_f")

                # c build: mch = -||e_k||^2 / 2, split hi+lo in f32r
                for kc in range(NKC):
                    ks = kc * KC
                    sq = sqp.tile([P, L // P, KC], F32R, tag="sq")
                    for lo in range(L // P):
                        nc.scalar.activation(sq[:, lo, :], et_t[:, lo, ks:ks + KC],
                                             AF.Square)
                    cps = ps_big.tile([P, KC], F32, tag="ps_dist")
                    for lo in range(L // P):
                        nc.tensor.matmul(cps[0:1, :], ones128_t[:], sq[:, lo, :],
                                         start=(lo == 0), stop=(lo == L // P - 1))
                    nc.scalar.activation(mch_f[:, ks:ks + KC], cps[0:1, :],
                                         AF.Copy, scale=-0.5)
                nc.vector.tensor_copy(mch_hi[:], mch_f[:])
                # lo = mch_f - mch_hi (bitcast hi to f32 for the subtract)
                nc.vector.tensor_sub(mch_f[:], mch_f[:], mch_hi[:].bitcast(F32))
                nc.vector.tensor_copy(mch_lo[:], mch_f[:])

                for i in range(NTILE):
                    ri = i * P
                    sp = vwork.tile([P, K], F32, tag="sp")
                    for kc in range(NKC):
                        ks = kc * KC
                        dps = ps_big.tile([P, KC], F32, tag="ps_dist")
                        nc.tensor.matmul(dps[:], zeT[:, 0, ri:ri + P],
                                         et_t[:, 0, ks:ks + KC], start=True, stop=False)
                        nc.tensor.matmul(dps[:], zeT[:, 1, ri:ri + P],
                                         et_t[:, 1, ks:ks + KC], start=False, stop=False)
                        nc.tensor.matmul(dps[:], ones1_t[:], mch_hi[:, ks:ks + KC],
                                         start=False, stop=False)
                        nc.tensor.matmul(dps[:], ones1_t[:], mch_lo[:, ks:ks + KC],
                                         start=False, stop=True)
                        nc.scalar.copy(sp[:, ks:ks + KC], dps[:])
                    mx8 = vwork.tile([P, 8], F32, tag="mx8")
                    ix8 = vwork.tile([P, 8], mybir.dt.uint32, tag="ix8")
                    nc.vector.max(mx8[:], sp[:])
                    nc.vector.max_index(ix8[:], mx8[:], sp[:])
                    ixf = vwork.tile([P, 1], F32, tag="ixf")
                    nc.vector.tensor_copy(ixf[:], ix8[:, 0:1])
                    ixu = vwork.tile([P, 1], mybir.dt.uint32, tag="ixu")
                    nc.vector.tensor_copy(ixu[:], ix8[:, 0:1])
                    # one-hot (int32) on gpsimd, one big DMA per row-tile
                    oh = ohp.tile([P, K], mybir.dt.int32, tag="oh")
                    for kc in range(NKC):
                        ks = kc * KC
                        nc.gpsimd.tensor_scalar(oh[:, ks:ks + KC],
                                                iota_t[:, ks:ks + KC],
                                                ixf[:], None, ALU.is_equal)
                    nc.sync.dma_start(oh_r[ri:ri + P, :], oh[:])
                    # gather z_q rows from HBM
                    zq = vwork.tile([P, L], F32, tag="zq")
                    nc.gpsimd.indirect_dma_start(
                        out=zq[:], out_offset=None, in_=ep_d.ap(),
                        in_offset=IndirectOffsetOnAxis(ap=ixu[:], axis=0),
                    )
                    # transpose to feature-major (rounded to f32r for the decoder)
                    for lo in range(L // P):
                        tps = ps_tp.tile([P, P], F32, tag="tp")
                        nc.tensor.transpose(tps[:], zq[:, lo * P:(lo + 1) * P], ident_t[:])
                        nc.scalar.copy(zqT[:, lo, ri:ri + P], tps[:])
                    # codebook-loss partial: sum((z_e - z_q)^2) for these rows
                    df = vwork.tile([P, L // P, P], F32, tag="df")
                    nc.gpsimd.tensor_tensor(
                        df[:], zeT[:, :, ri:ri + P].bitcast(F32),
                        zqT[:, :, ri:ri + P].bitcast(F32), ALU.subtract)
                    nc.scalar.activation(df[:], df[:], AF.Square,
                                         accum_out=s2buf[:, i:i + 1])

            # ================= P3: decoder =================
            with tc.tile_pool(name="decw", bufs=1) as decw, \
                 tc.tile_pool(name="decwork", bufs=2) as dwork, \
                 tc.tile_pool(name="decwork1", bufs=1) as dwork1:
                dw2_t = decw.tile([P, H // P, H], F32R, tag="dw2")
                dw3_t = decw.tile([P, H // P, D], F32R, tag="dw3")
                nc.sync.dma_start(dw2_t[:], dw2_r[:])
                nc.sync.dma_start(dw3_t[:], dw3_r[:])

                for st in range(NST):
                    rs = st * RBLK
                    g1 = dwork.tile([P, H // P, RBLK], F32R, tag="g1")
                    for f in range(H // P):
                        pt = ps_mm.tile([P, RBLK], F32, tag="ps_enc")
                        for d_ in range(L // P):
                            nc.tensor.matmul(
                                pt[:], dw1_t[:, d_, f * P:(f + 1) * P],
                                zqT[:, d_, rs:rs + RBLK],
                                start=(d_ == 0), stop=(d_ == L // P - 1),
                            )
                        nc.scalar.activation(g1[:, f, :], pt[:], AF.Gelu,
                                             bias=db1_t[:, f:f + 1])
                    g2 = dwork1.tile([P, H // P, RBLK], F32R, tag="g2")
                    for f in range(H // P):
                        pt = ps_mm.tile([P, RBLK], F32, tag="ps_enc")
                        for d_ in range(H // P):
                            nc.tensor.matmul(
                                pt[:], dw2_t[:, d_, f * P:(f + 1) * P], g1[:, d_, :],
                                start=(d_ == 0), stop=(d_ == H // P - 1),
                            )
                        nc.scalar.activation(g2[:, f, :], pt[:], AF.Gelu,
                                             bias=db2_t[:, f:f + 1])
                    xp = dwork.tile([P, D // P, RBLK], F32, tag="xp")
                    for f in range(D // P):
                        pt = ps_mm.tile([P, RBLK], F32, tag="ps_enc")
                        for d_ in range(H // P):
                            nc.tensor.matmul(
                                pt[:], dw3_t[:, d_, f * P:(f + 1) * P], g2[:, d_, :],
                                start=(d_ == 0), stop=(d_ == H // P - 1),
                            )
                        nc.scalar.activation(xp[:, f, :], pt[:], AF.Sigmoid,
                                             bias=db3_t[:, f:f + 1])
                    nc.sync.dma_start(xpT_r[st], xp[:])
                    # recon-loss partial: sum((x - x_pred)^2)
                    xtf = dwork.tile([P, D // P, RBLK], F32, tag="xtf")
                    nc.sync.dma_start(xtf[:], xT_r[st])
                    nc.vector.tensor_sub(xtf[:], xtf[:], xp[:])
                    nc.scalar.activation(xtf[:], xtf[:], AF.Square,
                                         accum_out=s1buf[:, st:st + 1])

            # ================= P4: loss partials out =================
            lp = persist.tile([P, 2], F32, tag="lossp")
            nc.vector.reduce_sum(lp[:, 0:1], s1buf[:], axis=mybir.AxisListType.X)
            nc.vector.reduce_sum(lp[:, 1:2], s2buf[:], axis=mybir.AxisListType.X)
            nc.sync.dma_start(lossp_d.ap(), lp[:])

    nc.finalize()
    return nc


def _wtile(w):
    """[K_in, F] -> [128, K_in//128, F] (partition-major chunks, contiguous
    per partition)."""
    w = np.asarray(w, np.float32)
    return np.ascontiguousarray(w.reshape(-1, P, w.shape[1]).transpose(1, 0, 2))


def _prep_shared(inputs):
    ep = np.ascontiguousarray(inputs["embed_pool"], dtype=np.float32)
    shared = {
        "eT": _wtile(np.ascontiguousarray(ep.T)),
        "ep": ep,
        "ew1": _wtile(inputs["ew1"]),
        "ew2": _wtile(inputs["ew2"]),
        "ew3": _wtile(inputs["ew3"]),
        "dw1": _wtile(inputs["dw1"]),
        "dw2": _wtile(inputs["dw2"]),
        "dw3": _wtile(inputs["dw3"]),
        "eb1": np.ascontiguousarray(np.asarray(inputs["eb1"], np.float32).reshape(-1, P).T),
        "eb2": np.ascontiguousarray(np.asarray(inputs["eb2"], np.float32).reshape(-1, P).T),
        "eb3": np.ascontiguousarray(np.asarray(inputs["eb3"], np.float32).reshape(-1, P).T),
        "db1": np.ascontiguousarray(np.asarray(inputs["db1"], np.float32).reshape(-1, P).T),
        "db2": np.ascontiguousarray(np.asarray(inputs["db2"], np.float32).reshape(-1, P).T),
        "db3": np.ascontiguousarray(np.asarray(inputs["db3"], np.float32).reshape(-1, P).T),
        "iota16": np.ascontiguousarray(
            np.broadcast_to(np.arange(K, dtype=np.int16), (P, K))),
        "ident": np.eye(P, dtype=np.float32),
        "ones1": np.ones((1, P), np.float32),
        "ones128": np.ones((P, 1), np.float32),
    }
    return shared


def _run(inputs, trace=False):
    if "nc" not in _cache:
        _cache["nc"] = _build()
    nc = _cache["nc"]
    return _run_nc(nc, inputs, trace)


def _run_nc(nc, inputs, trace=False):
    in_maps = _make_in_maps(inputs)
    res = run_bass_kernel_spmd(nc, in_maps, core_ids=list(range(NCORES)),
                               trace=trace)
    return _assemble(res.results), res


def _assemble(results):
    x_pred = np.empty((N, D), np.float32)
    z_disc = np.empty((N, K), np.int32)
    s1 = 0.0
    s2 = 0.0
    for c, r in enumerate(results):
        x_pred[c * NS:(c + 1) * NS] = _xuntile(r["xpredT"])
        z_disc[c * NS:(c + 1) * NS] = r["onehot"]
        s1 += r["lossp"][:, 0].astype(np.float64).sum()
        s2 += r["lossp"][:, 1].astype(np.float64).sum()
    loss = np.float32((s1 + 1.25 * s2) / N)
    return (x_pred, z_disc, loss)


def kernel(**inputs):
    out, _ = _run(inputs, trace=False)
    return out


def _bench_nc(nc, in_maps, iters):
    """Build the sharded jit once for `nc`, keep inputs device-resident,
    re-donate outputs; return (times, host_outs_of_last_iter)."""
    import time

    import jax
    from jax.sharding import Mesh, NamedSharding, PartitionSpec
    from jax.experimental.shard_map import shard_map

    from concourse import bass2jax as B2J

    B2J.install_neuronx_cc_hook()
    partition_name = nc.partition_id_tensor.name if nc.partition_id_tensor else None
    in_names, out_names, out_avals, zero_outs = [], [], [], []
    for alloc in nc.m.functions[0].allocations:
        if not isinstance(alloc, mybir.MemoryLocationSet):
            continue
        name = alloc.memorylocations[0].name
        if alloc.kind == "ExternalInput":
            if name != partition_name:
                in_names.append(name)
        elif alloc.kind == "ExternalOutput":
            out_names.append(name)
            shape = tuple(alloc.tensor_shape)
            dtype = mybir.dt.np(alloc.dtype)
            out_avals.append(jax.core.ShapedArray(shape, dtype))
            zero_outs.append(np.zeros(shape, dtype))
    n_params = len(in_names)
    n_outs = len(out_avals)
    in_names_all = in_names + out_names + ([partition_name] if partition_name else [])
    donate = tuple(range(n_params, n_params + n_outs))

    def _body(*args):
        operands = list(args)
        if partition_name is not None:
            operands.append(B2J.partition_id_tensor())
        return tuple(B2J._bass_exec_p.bind(
            *operands, out_avals=tuple(out_avals), in_names=tuple(in_names_all),
            out_names=tuple(out_names), lowering_input_output_aliases=(),
            sim_require_finite=True, sim_require_nnan=True, nc=nc))

    devices = jax.devices()[:NCORES]
    mesh = Mesh(np.asarray(devices), ("core",))
    sharded = jax.jit(
        shard_map(_body, mesh=mesh,
                  in_specs=(PartitionSpec("core"),) * (n_params + n_outs),
                  out_specs=(PartitionSpec("core"),) * n_outs, check_rep=False),
        donate_argnums=donate, keep_unused=True)

    sh = NamedSharding(mesh, PartitionSpec("core"))
    concat_in = [
        jax.device_put(
            np.concatenate([np.asarray(in_maps[c][nm]) for c in range(NCORES)], 0), sh)
        for nm in in_names
    ]
    concat_zeros = [
        jax.device_put(np.zeros((NCORES * z.shape[0], *z.shape[1:]), z.dtype), sh)
        for z in zero_outs
    ]
    outs = sharded(*concat_in, *concat_zeros)
    jax.block_until_ready(outs)
    times = []
    for _ in range(iters):
        t0 = time.perf_counter()
        outs = sharded(*concat_in, *outs)
        jax.block_until_ready(outs)
        times.append(time.perf_counter() - t0)
    host_outs = [
        {nm: np.asarray(outs[i]).reshape(NCORES, *out_avals[i].shape)[c]
         for i, nm in enumerate(out_names)}
        for c in range(NCORES)
    ]
    return times, host_outs


def _xtile(x_shard):
    """[NS, D] -> [NST, 128, D//128, RBLK]: xtile[st, p, o, r] =
    x[st*RBLK + r, o*128 + p]."""
    v = x_shard.reshape(NST, RBLK, D // P, P)
    return np.ascontiguousarray(v.transpose(0, 3, 2, 1))


def _xuntile(xt):
    """Inverse of _xtile: [NST, 128, D//128, RBLK] -> [NS, D]."""
    return np.ascontiguousarray(
        xt.transpose(0, 3, 2, 1).reshape(NS, D))


def _make_in_maps(inputs):
    x = np.ascontiguousarray(np.asarray(inputs["x"], np.float32))
    shared = _prep_shared(inputs)
    in_maps = []
    for c in range(NCORES):
        m = dict(shared)
        m["xT"] = _xtile(x[c * NS:(c + 1) * NS])
        in_maps.append(m)
    return in_maps


def bench(inputs, iters=8, repeat=65):
    """Two-point measurement: time repeat=1 and repeat=R NEFFs in the same
    session; device time = (min(tR) - min(t1)) / (R - 1). min is robust to
    the bimodal axon dispatch-overhead noise."""
    in_maps = _make_in_maps(inputs)
    if "nc" not in _cache:
        _cache["nc"] = _build()
    if ("ncR", repeat) not in _cache:
        _cache[("ncR", repeat)] = _build(repeat=repeat)
    t1, host_outs = _bench_nc(_cache["nc"], in_maps, iters)
    tR, host_outs_R = _bench_nc(_cache[("ncR", repeat)], in_maps, iters)
    dev = (min(tR) - min(t1)) / (repeat - 1)
    return dev, t1, tR, host_outs, host_outs_R


# revision 36
# speedup vs baseline: 2.5732x; 2.5732x over previous
"""MinVQVAE1D forward pass on 8 Trainium2 NeuronCores.

Data-parallel: batch N=16384 sharded 2048 rows/core; codebook + MLP weights
replicated. All matmuls run in float32r (fp32 storage, TF32-like PE mode at
bf16 speed). The VQ argmin is computed as argmax of (z_e . e_k - ||e_k||^2/2)
where the -c/2 term is folded into the PE accumulation as two K=1 ones-row
matmuls (hi+lo split so the c term keeps ~fp32 accuracy). Index extraction
uses the DVE max/max_index top-8 instructions; z_q rows come back via an
indirect-DMA gather from HBM. Loss partial sums are produced on-device and
finished on host.

Self-contained: hardcodes all shapes from the problem spec.
"""
import sys

sys.path.insert(0, "/opt/trn_rl_repo")

import numpy as np

import concourse.bass as bass
import concourse.mybir as mybir
import concourse.tile as tile
from concourse import bacc
from concourse.bass import IndirectOffsetOnAxis
from concourse.bass_utils import run_bass_kernel_spmd

# problem shapes
N, D, H, L, K = 16384, 1024, 1024, 256, 4096
NCORES = 8
NS = N // NCORES          # rows per core
P = 128
RBLK = 256                # supertile row block (moving free dim; >=256 keeps f32r at 1 cyc/row)
NST = NS // RBLK          # supertiles per core
NTILE = NS // P           # 128-row tiles per core (VQ phase)
KC = 512                  # distance k-chunk (one PSUM bank)
NKC = K // KC

F32 = mybir.dt.float32
F32R = mybir.dt.float32r
AF = mybir.ActivationFunctionType
ALU = mybir.AluOpType

_cache = {}


def _build(repeat=1, phases=(1, 2, 3), p2_level=5):
    nc = bacc.Bacc(None, target_bir_lowering=False)

    # ---- DRAM I/O ----
    # xT pre-tiled on host: [NST, 128, D//128, RBLK]; one contiguous-per-
    # partition DMA per supertile (128 descriptors x 8KB).
    xT_d = nc.dram_tensor("xT", [NST, P, D // P, RBLK], F32, kind="ExternalInput")
    eT_d = nc.dram_tensor("eT", [P, L // P, K], F32R, kind="ExternalInput")
    ep_d = nc.dram_tensor("ep", [K, L], F32, kind="ExternalInput")
    ew1_d = nc.dram_tensor("ew1", [P, D // P, H], F32R, kind="ExternalInput")
    ew2_d = nc.dram_tensor("ew2", [P, H // P, H], F32R, kind="ExternalInput")
    ew3_d = nc.dram_tensor("ew3", [P, H // P, L], F32R, kind="ExternalInput")
    dw1_d = nc.dram_tensor("dw1", [P, L // P, H], F32R, kind="ExternalInput")
    dw2_d = nc.dram_tensor("dw2", [P, H // P, H], F32R, kind="ExternalInput")
    dw3_d = nc.dram_tensor("dw3", [P, H // P, D], F32R, kind="ExternalInput")
    # biases pre-shaped [128, nchunks] on host
    eb1_d = nc.dram_tensor("eb1", [P, H // P], F32, kind="ExternalInput")
    eb2_d = nc.dram_tensor("eb2", [P, H // P], F32, kind="ExternalInput")
    eb3_d = nc.dram_tensor("eb3", [P, L // P], F32, kind="ExternalInput")
    db1_d = nc.dram_tensor("db1", [P, H // P], F32, kind="ExternalInput")
    db2_d = nc.dram_tensor("db2", [P, H // P], F32, kind="ExternalInput")
    db3_d = nc.dram_tensor("db3", [P, D // P], F32, kind="ExternalInput")
    p4096_d = nc.dram_tensor("p4096", [P, 1], F32, kind="ExternalInput")
    ident_d = nc.dram_tensor("ident", [P, P], F32, kind="ExternalInput")
    ones1_d = nc.dram_tensor("ones1", [1, P], F32R, kind="ExternalInput")
    ones128_d = nc.dram_tensor("ones128", [P, 1], F32R, kind="ExternalInput")

    xpT_d = nc.dram_tensor("xpredT", [NST, P, D // P, RBLK], F32, kind="ExternalOutput")
    oh_d = nc.dram_tensor("onehot", [NS, K], mybir.dt.int32, kind="ExternalOutput")
    lossp_d = nc.dram_tensor("lossp", [P, 2], F32, kind="ExternalOutput")

    xT_r = xT_d.ap()
    xpT_r = xpT_d.ap()
    ew1_r = ew1_d.ap()
    ew2_r = ew2_d.ap()
    ew3_r = ew3_d.ap()
    dw1_r = dw1_d.ap()
    dw2_r = dw2_d.ap()
    dw3_r = dw3_d.ap()
    eT_r = eT_d.ap()
    oh_r = oh_d.ap()

    with tile.TileContext(nc) as tc:
        import contextlib

        stack = contextlib.ExitStack()
        with stack:
            persist = stack.enter_context(tc.tile_pool(name="persist", bufs=1))
            ps_mm = stack.enter_context(tc.tile_pool(name="ps_mm", bufs=3, space="PSUM"))
            ps_big = stack.enter_context(tc.tile_pool(name="ps_big", bufs=3, space="PSUM"))
            ps_tp = stack.enter_context(tc.tile_pool(name="ps_tp", bufs=2, space="PSUM"))

            # ---- persistent small tensors ----
            dw1_t = persist.tile([P, L // P, H], F32R, tag="dw1")
            nc.sync.dma_start(dw1_t[:], dw1_r[:])
            eb1_t = persist.tile([P, H // P], F32, tag="eb1")
            eb2_t = persist.tile([P, H // P], F32, tag="eb2")
            eb3_t = persist.tile([P, L // P], F32, tag="eb3")
            db1_t = persist.tile([P, H // P], F32, tag="db1")
            db2_t = persist.tile([P, H // P], F32, tag="db2")
            db3_t = persist.tile([P, D // P], F32, tag="db3")
            for t, d in [(eb1_t, eb1_d), (eb2_t, eb2_d), (eb3_t, eb3_d),
                         (db1_t, db1_d), (db2_t, db2_d), (db3_t, db3_d)]:
                nc.sync.dma_start(t[:], d.ap())
            ident_t = persist.tile([P, P], F32, tag="ident")
            nc.sync.dma_start(ident_t[:], ident_d.ap())
            ones1_t = persist.tile([1, P], F32R, tag="ones1")
            nc.sync.dma_start(ones1_t[:], ones1_d.ap())
            ones128_t = persist.tile([P, 1], F32R, tag="ones128")
            nc.sync.dma_start(ones128_t[:], ones128_d.ap())
            mch_hi = persist.tile([1, K], F32R, tag="mch_hi")
            mch_lo = persist.tile([1, K], F32R, tag="mch_lo")
            zeT = persist.tile([P, L // P, NS], F32R, tag="zeT")    # 2 MB
            zqT = persist.tile([P, L // P, NS], F32R, tag="zqT")    # 2 MB
            s1buf = persist.tile([P, NST], F32, tag="s1buf")
            s2buf = persist.tile([P, NTILE], F32, tag="s2buf")
            nc.vector.memset(s1buf[:], 0.0)
            nc.vector.memset(s2buf[:], 0.0)

            if repeat > 1:
                stack.enter_context(tc.For_i(0, repeat, 1))

            # ================= P1: encoder =================
            if 1 in phases:
              with tc.tile_pool(name="encw", bufs=1) as encw, \
                 tc.tile_pool(name="encwork", bufs=2) as work:
                ew1_t = encw.tile([P, D // P, H], F32R, tag="ew1")
                ew2_t = encw.tile([P, H // P, H], F32R, tag="ew2")
                ew3_t = encw.tile([P, H // P, L], F32R, tag="ew3")
                nc.sync.dma_start(ew1_t[:], ew1_r[:])
                nc.sync.dma_start(ew2_t[:], ew2_r[:])
                nc.sync.dma_start(ew3_t[:], ew3_r[:])

                for st in range(NST):
                    rs = st * RBLK
                    xt = work.tile([P, D // P, RBLK], F32R, tag="xt")
                    nc.sync.dma_start(xt[:], xT_r[st].bitcast(F32R))
                    h1 = work.tile([P, H // P, RBLK], F32R, tag="h1")
                    for f in range(H // P):
                        pt = ps_mm.tile([P, RBLK], F32, tag="ps_enc")
                        for d_ in range(D // P):
                            nc.tensor.matmul(
                                pt[:], ew1_t[:, d_, f * P:(f + 1) * P], xt[:, d_, :],
                                start=(d_ == 0), stop=(d_ == D // P - 1),
                            )
                        nc.scalar.activation(h1[:, f, :], pt[:], AF.Gelu,
                                             bias=eb1_t[:, f:f + 1])
                    h2 = work.tile([P, H // P, RBLK], F32R, tag="h2")
                    for f in range(H // P):
                        pt = ps_mm.tile([P, RBLK], F32, tag="ps_enc")
                        for d_ in range(H // P):
                            nc.tensor.matmul(
                                pt[:], ew2_t[:, d_, f * P:(f + 1) * P], h1[:, d_, :],
                                start=(d_ == 0), stop=(d_ == H // P - 1),
                            )
                        nc.scalar.activation(h2[:, f, :], pt[:], AF.Gelu,
                                             bias=eb2_t[:, f:f + 1])
                    for f in range(L // P):
                        pt = ps_mm.tile([P, RBLK], F32, tag="ps_enc")
                        for d_ in range(H // P):
                            nc.tensor.matmul(
                                pt[:], ew3_t[:, d_, f * P:(f + 1) * P], h2[:, d_, :],
                                start=(d_ == 0), stop=(d_ == H // P - 1),
                            )
                        nc.scalar.activation(zeT[:, f, rs:rs + RBLK], pt[:],
                                             AF.Identity, bias=eb3_t[:, f:f + 1])

            # ================= P2: VQ =================
            if 2 in phases:
              with tc.tile_pool(name="vq", bufs=1) as vq, \
                 tc.tile_pool(name="vqwork", bufs=2) as vwork, \
                 tc.tile_pool(name="sqpool", bufs=2) as sqp:
                et_t = vq.tile([P, L // P, K], F32R, tag="et")
                nc.sync.dma_start(et_t[:], eT_r[:])
                p4096_t = vq.tile([P, 1], F32, tag="p4096")
                nc.sync.dma_start(p4096_t[:], p4096_d.ap())
                ones_i32 = vq.tile([P, 1], mybir.dt.int32, tag="ones_i32")
                nc.vector.memset(ones_i32[:], 1)
                ixu_all = vq.tile([P, NTILE], mybir.dt.uint32, tag="ixu_all")
                zq_all = vq.tile([P, NTILE, L], F32, tag="zq_all")
                oh_flat = oh_d.ap().rearrange("a b -> (a b)").unsqueeze(1)
                mch_f = vq.tile([1, K], F32, tag="mch_f")

                # c build: mch = -||e_k||^2 / 2, split hi+lo in f32r
                for kc in range(NKC):
                    ks = kc * KC
                    sq = sqp.tile([P, L // P, KC], F32R, tag="sq")
                    for lo in range(L // P):
                        nc.scalar.activation(sq[:, lo, :], et_t[:, lo, ks:ks + KC],
                                             AF.Square)
                    cps = ps_big.tile([P, KC], F32, tag="ps_dist")
                    for lo in range(L // P):
                        nc.tensor.matmul(cps[0:1, :], ones128_t[:], sq[:, lo, :],
                                         start=(lo == 0), stop=(lo == L // P - 1))
                    nc.scalar.activation(mch_f[:, ks:ks + KC], cps[0:1, :],
                                         AF.Copy, scale=-0.5)
                nc.vector.tensor_copy(mch_hi[:], mch_f[:])
                # lo = mch_f - mch_hi (bitcast hi to f32 for the subtract)
                nc.vector.tensor_sub(mch_f[:], mch_f[:], mch_hi[:].bitcast(F32))
                nc.vector.tensor_copy(mch_lo[:], mch_f[:])

                for i in range(NTILE):
                    ri = i * P
                    sp = vwork.tile([P, K], F32, tag="sp")
                    for kc in range(NKC):
                        ks = kc * KC
                        dps = ps_big.tile([P, KC], F32, tag="ps_dist")
                        nc.tensor.matmul(dps[:], zeT[:, 0, ri:ri + P],
                                         et_t[:, 0, ks:ks + KC], start=True, stop=False)
                        nc.tensor.matmul(dps[:], zeT[:, 1, ri:ri + P],
                                         et_t[:, 1, ks:ks + KC], start=False, stop=False)
                        nc.tensor.matmul(dps[:], ones1_t[:], mch_hi[:, ks:ks + KC],
                                         start=False, stop=False)
                        nc.tensor.matmul(dps[:], ones1_t[:], mch_lo[:, ks:ks + KC],
                                         start=False, stop=True)
                        nc.scalar.copy(sp[:, ks:ks + KC], dps[:])
                    if p2_level < 2:
                        continue
                    mx8 = vwork.tile([P, 8], F32, tag="mx8")
                    ix8 = vwork.tile([P, 8], mybir.dt.uint32, tag="ix8")
                    nc.vector.max(mx8[:], sp[:])
                    nc.vector.max_index(ix8[:], mx8[:], sp[:])
                    ixf = vwork.tile([P, 1], F32, tag="ixf")
                    nc.vector.tensor_copy(ixf[:], ix8[:, 0:1])
                    nc.vector.tensor_copy(ixu_all[:, i:i + 1], ix8[:, 0:1])
                    if p2_level >= 3:
                        # z_discrete: scatter 128 int32 ones at flat offsets
                        # (output DRAM is zero-initialized by the runtime)
                        off_f = vwork.tile([P, 1], F32, tag="off_f")
                        nc.vector.tensor_scalar(off_f[:], ixf[:], p4096_t[:],
                                                float(ri * K), ALU.add, ALU.add)
                        off_i = vwork.tile([P, 1], mybir.dt.int32, tag="off_i")
                        nc.vector.tensor_copy(off_i[:], off_f[:])
                        nc.gpsimd.indirect_dma_start(
                            out=oh_flat, out_offset=IndirectOffsetOnAxis(
                                ap=off_i[:], axis=0),
                            in_=ones_i32[:], in_offset=None,
                        )
                    if p2_level >= 4:
                        # gather z_q rows from HBM
                        nc.gpsimd.indirect_dma_start(
                            out=zq_all[:, i, :], out_offset=None, in_=ep_d.ap(),
                            in_offset=IndirectOffsetOnAxis(
                                ap=ixu_all[:, i:i + 1], axis=0),
                        )
                if p2_level >= 5:
                    for i in range(NTILE):
                        ri = i * P
                        # transpose to feature-major (f32r for the decoder)
                        for lo in range(L // P):
                            tps = ps_tp.tile([P, P], F32, tag="tp")
                            nc.tensor.transpose(
                                tps[:], zq_all[:, i, lo * P:(lo + 1) * P],
                                ident_t[:])
                            nc.scalar.copy(zqT[:, lo, ri:ri + P], tps[:])
                        # codebook-loss partial: sum((z_e - z_q)^2)
                        df = vwork.tile([P, L // P, P], F32, tag="df")
                        nc.vector.tensor_sub(
                            df[:], zeT[:, :, ri:ri + P].bitcast(F32),
                            zqT[:, :, ri:ri + P].bitcast(F32))
                        nc.scalar.activation(df[:], df[:], AF.Square,
                                             accum_out=s2buf[:, i:i + 1])

            # ================= P3: decoder =================
            if 3 in phases:
              with tc.tile_pool(name="decw", bufs=1) as decw, \
                 tc.tile_pool(name="decwork", bufs=2) as dwork, \
                 tc.tile_pool(name="decwork1", bufs=1) as dwork1:
                dw2_t = decw.tile([P, H // P, H], F32R, tag="dw2")
                dw3_t = decw.tile([P, H // P, D], F32R, tag="dw3")
                nc.sync.dma_start(dw2_t[:], dw2_r[:])
                nc.sync.dma_start(dw3_t[:], dw3_r[:])

                for st in range(NST):
                    rs = st * RBLK
                    g1 = dwork.tile([P, H // P, RBLK], F32R, tag="g1")
                    for f in range(H // P):
                        pt = ps_mm.tile([P, RBLK], F32, tag="ps_enc")
                        for d_ in range(L // P):
                            nc.tensor.matmul(
                                pt[:], dw1_t[:, d_, f * P:(f + 1) * P],
                                zqT[:, d_, rs:rs + RBLK],
                                start=(d_ == 0), stop=(d_ == L // P - 1),
                            )
                        nc.scalar.activation(g1[:, f, :], pt[:], AF.Gelu,
                                             bias=db1_t[:, f:f + 1])
                    g2 = dwork1.tile([P, H // P, RBLK], F32R, tag="g2")
                    for f in range(H // P):
                        pt = ps_mm.tile([P, RBLK], F32, tag="ps_enc")
                        for d_ in range(H // P):
                            nc.tensor.matmul(
                                pt[:], dw2_t[:, d_, f * P:(f + 1) * P], g1[:, d_, :],
                                start=(d_ == 0), stop=(d_ == H // P - 1),
                            )
                        nc.scalar.activation(g2[:, f, :], pt[:], AF.Gelu,
                                             bias=db2_t[:, f:f + 1])
                    xp = dwork.tile([P, D // P, RBLK], F32, tag="xp")
                    for f in range(D // P):
                        pt = ps_mm.tile([P, RBLK], F32, tag="ps_enc")
                        for d_ in range(H // P):
                            nc.tensor.matmul(
                                pt[:], dw3_t[:, d_, f * P:(f + 1) * P], g2[:, d_, :],
                                start=(d_ == 0), stop=(d_ == H // P - 1),
                            )
                        nc.scalar.activation(xp[:, f, :], pt[:], AF.Sigmoid,
                                             bias=db3_t[:, f:f + 1])
                    nc.sync.dma_start(xpT_r[st], xp[:])
                    # recon-loss partial: sum((x - x_pred)^2)
                    xtf = dwork.tile([P, D // P, RBLK], F32, tag="xtf")
                    nc.sync.dma_start(xtf[:], xT_r[st])
                    nc.vector.tensor_sub(xtf[:], xtf[:], xp[:])
                    nc.scalar.activation(xtf[:], xtf[:], AF.Square,
                                         accum_out=s1buf[:, st:st + 1])

            # ================= P4: loss partials out =================
            lp = persist.tile([P, 2], F32, tag="lossp")
            nc.vector.reduce_sum(lp[:, 0:1], s1buf[:], axis=mybir.AxisListType.X)
            nc.vector.reduce_sum(lp[:, 1:2], s2buf[:], axis=mybir.AxisListType.X)
            nc.sync.dma_start(lossp_d.ap(), lp[:])

    nc.finalize()
    return nc


def _wtile(w):
    """[K_in, F] -> [128, K_in//128, F] (partition-major chunks, contiguous
    per partition)."""
    w = np.asarray(w, np.float32)
    return np.ascontiguousarray(w.reshape(-1, P, w.shape[1]).transpose(1, 0, 2))


def _prep_shared(inputs):
    ep = np.ascontiguousarray(inputs["embed_pool"], dtype=np.float32)
    shared = {
        "eT": _wtile(np.ascontiguousarray(ep.T)),
        "ep": ep,
        "ew1": _wtile(inputs["ew1"]),
        "ew2": _wtile(inputs["ew2"]),
        "ew3": _wtile(inputs["ew3"]),
        "dw1": _wtile(inputs["dw1"]),
        "dw2": _wtile(inputs["dw2"]),
        "dw3": _wtile(inputs["dw3"]),
        "eb1": np.ascontiguousarray(np.asarray(inputs["eb1"], np.float32).reshape(-1, P).T),
        "eb2": np.ascontiguousarray(np.asarray(inputs["eb2"], np.float32).reshape(-1, P).T),
        "eb3": np.ascontiguousarray(np.asarray(inputs["eb3"], np.float32).reshape(-1, P).T),
        "db1": np.ascontiguousarray(np.asarray(inputs["db1"], np.float32).reshape(-1, P).T),
        "db2": np.ascontiguousarray(np.asarray(inputs["db2"], np.float32).reshape(-1, P).T),
        "db3": np.ascontiguousarray(np.asarray(inputs["db3"], np.float32).reshape(-1, P).T),
        "p4096": (np.arange(P, dtype=np.float32) * K).reshape(P, 1),
        "ident": np.eye(P, dtype=np.float32),
        "ones1": np.ones((1, P), np.float32),
        "ones128": np.ones((P, 1), np.float32),
    }
    return shared


def _run(inputs, trace=False):
    if "nc" not in _cache:
        _cache["nc"] = _build()
    nc = _cache["nc"]
    return _run_nc(nc, inputs, trace)


def _run_nc(nc, inputs, trace=False):
    in_maps = _make_in_maps(inputs)
    res = run_bass_kernel_spmd(nc, in_maps, core_ids=list(range(NCORES)),
                               trace=trace)
    return _assemble(res.results), res


def _assemble(results):
    x_pred = np.empty((N, D), np.float32)
    z_disc = np.empty((N, K), np.int32)
    s1 = 0.0
    s2 = 0.0
    for c, r in enumerate(results):
        x_pred[c * NS:(c + 1) * NS] = _xuntile(r["xpredT"])
        z_disc[c * NS:(c + 1) * NS] = r["onehot"]
        s1 += r["lossp"][:, 0].astype(np.float64).sum()
        s2 += r["lossp"][:, 1].astype(np.float64).sum()
    loss = np.float32((s1 + 1.25 * s2) / N)
    return (x_pred, z_disc, loss)


def kernel(**inputs):
    out, _ = _run(inputs, trace=False)
    return out


def _bench_nc(nc, in_maps, iters):
    """Build the sharded jit once for `nc`, keep inputs device-resident,
    re-donate outputs; return (times, host_outs_of_last_iter)."""
    import time

    import jax
    from jax.sharding import Mesh, NamedSharding, PartitionSpec
    from jax.experimental.shard_map import shard_map

    from concourse import bass2jax as B2J

    B2J.install_neuronx_cc_hook()
    partition_name = nc.partition_id_tensor.name if nc.partition_id_tensor else None
    in_names, out_names, out_avals, zero_outs = [], [], [], []
    for alloc in nc.m.functions[0].allocations:
        if not isinstance(alloc, mybir.MemoryLocationSet):
            continue
        name = alloc.memorylocations[0].name
        if alloc.kind == "ExternalInput":
            if name != partition_name:
                in_names.append(name)
        elif alloc.kind == "ExternalOutput":
            out_names.append(name)
            shape = tuple(alloc.tensor_shape)
            dtype = mybir.dt.np(alloc.dtype)
            out_avals.append(jax.core.ShapedArray(shape, dtype))
            zero_outs.append(np.zeros(shape, dtype))
    n_params = len(in_names)
    n_outs = len(out_avals)
    in_names_all = in_names + out_names + ([partition_name] if partition_name else [])
    donate = tuple(range(n_params, n_params + n_outs))

    def _body(*args):
        operands = list(args)
        if partition_name is not None:
            operands.append(B2J.partition_id_tensor())
        return tuple(B2J._bass_exec_p.bind(
            *operands, out_avals=tuple(out_avals), in_names=tuple(in_names_all),
            out_names=tuple(out_names), lowering_input_output_aliases=(),
            sim_require_finite=True, sim_require_nnan=True, nc=nc))

    devices = jax.devices()[:NCORES]
    mesh = Mesh(np.asarray(devices), ("core",))
    sharded = jax.jit(
        shard_map(_body, mesh=mesh,
                  in_specs=(PartitionSpec("core"),) * (n_params + n_outs),
                  out_specs=(PartitionSpec("core"),) * n_outs, check_rep=False),
        donate_argnums=donate, keep_unused=True)

    sh = NamedSharding(mesh, PartitionSpec("core"))
    concat_in = [
        jax.device_put(
            np.concatenate([np.asarray(in_maps[c][nm]) for c in range(NCORES)], 0), sh)
        for nm in in_names
    ]
    concat_zeros = [
        jax.device_put(np.zeros((NCORES * z.shape[0], *z.shape[1:]), z.dtype), sh)
        for z in zero_outs
    ]
    outs = sharded(*concat_in, *concat_zeros)
    jax.block_until_ready(outs)
    times = []
    for _ in range(iters):
        t0 = time.perf_counter()
        outs = sharded(*concat_in, *outs)
        jax.block_until_ready(outs)
        times.append(time.perf_counter() - t0)
    host_outs = [
        {nm: np.asarray(outs[i]).reshape(NCORES, *out_avals[i].shape)[c]
         for i, nm in enumerate(out_names)}
        for c in range(NCORES)
    ]
    return times, host_outs


def _xtile(x_shard):
    """[NS, D] -> [NST, 128, D//128, RBLK]: xtile[st, p, o, r] =
    x[st*RBLK + r, o*128 + p]."""
    v = x_shard.reshape(NST, RBLK, D // P, P)
    return np.ascontiguousarray(v.transpose(0, 3, 2, 1))


def _xuntile(xt):
    """Inverse of _xtile: [NST, 128, D//128, RBLK] -> [NS, D]."""
    return np.ascontiguousarray(
        xt.transpose(0, 3, 2, 1).reshape(NS, D))


def _make_in_maps(inputs):
    x = np.ascontiguousarray(np.asarray(inputs["x"], np.float32))
    shared = _prep_shared(inputs)
    in_maps = []
    for c in range(NCORES):
        m = dict(shared)
        m["xT"] = _xtile(x[c * NS:(c + 1) * NS])
        in_maps.append(m)
    return in_maps


def bench(inputs, iters=8, repeat=65):
    """Two-point measurement: time repeat=1 and repeat=R NEFFs in the same
    session; device time = (min(tR) - min(t1)) / (R - 1). min is robust to
    the bimodal axon dispatch-overhead noise."""
    in_maps = _make_in_maps(inputs)
    if "nc" not in _cache:
        _cache["nc"] = _build()
    if ("ncR", repeat) not in _cache:
        _cache[("ncR", repeat)] = _build(repeat=repeat)
    t1, host_outs = _bench_nc(_cache["nc"], in_maps, iters)
    tR, host_outs_R = _bench_nc(_cache[("ncR", repeat)], in_maps, iters)
    dev = (min(tR) - min(t1)) / (repeat - 1)
    return dev, t1, tR, host_outs, host_outs_R


# revision 41
# speedup vs baseline: 2.6671x; 1.0365x over previous
"""MinVQVAE1D forward pass on 8 Trainium2 NeuronCores.

Data-parallel: batch N=16384 sharded 2048 rows/core; codebook + MLP weights
replicated. All matmuls run in float32r (fp32 storage, TF32-like PE mode at
bf16 speed). The VQ argmin is computed as argmax of (z_e . e_k - ||e_k||^2/2)
where the -c/2 term is folded into the PE accumulation as two K=1 ones-row
matmuls (hi+lo split so the c term keeps ~fp32 accuracy). Index extraction
uses the DVE max/max_index top-8 instructions; z_q rows come back via an
indirect-DMA gather from HBM. Loss partial sums are produced on-device and
finished on host.

Self-contained: hardcodes all shapes from the problem spec.
"""
import sys

sys.path.insert(0, "/opt/trn_rl_repo")

import numpy as np

import concourse.bass as bass
import concourse.mybir as mybir
import concourse.tile as tile
from concourse import bacc
from concourse.bass import IndirectOffsetOnAxis
from concourse.bass_utils import run_bass_kernel_spmd

# problem shapes
N, D, H, L, K = 16384, 1024, 1024, 256, 4096
NCORES = 8
NS = N // NCORES          # rows per core
P = 128
RBLK = 256                # supertile row block (moving free dim; >=256 keeps f32r at 1 cyc/row)
NST = NS // RBLK          # supertiles per core
NTILE = NS // P           # 128-row tiles per core (VQ phase)
KC = 512                  # distance k-chunk (one PSUM bank)
NKC = K // KC

F32 = mybir.dt.float32
F32R = mybir.dt.float32r
AF = mybir.ActivationFunctionType
ALU = mybir.AluOpType

_cache = {}


def _build(repeat=1, phases=(1, 2, 3), p2_level=5):
    nc = bacc.Bacc(None, target_bir_lowering=False)

    # ---- DRAM I/O ----
    # xT pre-tiled on host: [NST, 128, D//128, RBLK]; one contiguous-per-
    # partition DMA per supertile (128 descriptors x 8KB).
    xT_d = nc.dram_tensor("xT", [NST, P, D // P, RBLK], F32, kind="ExternalInput")
    eT_d = nc.dram_tensor("eT", [P, L // P, K], F32R, kind="ExternalInput")
    ep_d = nc.dram_tensor("ep", [K, L], F32, kind="ExternalInput")
    ew1_d = nc.dram_tensor("ew1", [P, D // P, H], F32R, kind="ExternalInput")
    ew2_d = nc.dram_tensor("ew2", [P, H // P, H], F32R, kind="ExternalInput")
    ew3_d = nc.dram_tensor("ew3", [P, H // P, L], F32R, kind="ExternalInput")
    dw1_d = nc.dram_tensor("dw1", [P, L // P, H], F32R, kind="ExternalInput")
    dw2_d = nc.dram_tensor("dw2", [P, H // P, H], F32R, kind="ExternalInput")
    dw3_d = nc.dram_tensor("dw3", [P, H // P, D], F32R, kind="ExternalInput")
    # biases pre-shaped [128, nchunks] on host
    eb1_d = nc.dram_tensor("eb1", [P, H // P], F32, kind="ExternalInput")
    eb2_d = nc.dram_tensor("eb2", [P, H // P], F32, kind="ExternalInput")
    eb3_d = nc.dram_tensor("eb3", [P, L // P], F32, kind="ExternalInput")
    db1_d = nc.dram_tensor("db1", [P, H // P], F32, kind="ExternalInput")
    db2_d = nc.dram_tensor("db2", [P, H // P], F32, kind="ExternalInput")
    db3_d = nc.dram_tensor("db3", [P, D // P], F32, kind="ExternalInput")
    p4096_d = nc.dram_tensor("p4096", [P, 1], F32, kind="ExternalInput")
    ident_d = nc.dram_tensor("ident", [P, P], F32, kind="ExternalInput")
    ones1_d = nc.dram_tensor("ones1", [1, P], F32R, kind="ExternalInput")
    ones128_d = nc.dram_tensor("ones128", [P, 1], F32R, kind="ExternalInput")

    xpT_d = nc.dram_tensor("xpredT", [NST, P, D // P, RBLK], F32, kind="ExternalOutput")
    oh_d = nc.dram_tensor("onehot", [NS, K], mybir.dt.int32, kind="ExternalOutput")
    lossp_d = nc.dram_tensor("lossp", [P, 2], F32, kind="ExternalOutput")

    xT_r = xT_d.ap()
    xpT_r = xpT_d.ap()
    ew1_r = ew1_d.ap()
    ew2_r = ew2_d.ap()
    ew3_r = ew3_d.ap()
    dw1_r = dw1_d.ap()
    dw2_r = dw2_d.ap()
    dw3_r = dw3_d.ap()
    eT_r = eT_d.ap()
    oh_r = oh_d.ap()

    with tile.TileContext(nc) as tc:
        import contextlib

        stack = contextlib.ExitStack()
        with stack:
            persist = stack.enter_context(tc.tile_pool(name="persist", bufs=1))
            ps_mm = stack.enter_context(tc.tile_pool(name="ps_mm", bufs=3, space="PSUM"))
            ps_big = stack.enter_context(tc.tile_pool(name="ps_big", bufs=3, space="PSUM"))
            ps_tp = stack.enter_context(tc.tile_pool(name="ps_tp", bufs=2, space="PSUM"))

            # ---- persistent small tensors ----
            dw1_t = persist.tile([P, L // P, H], F32R, tag="dw1")
            nc.sync.dma_start(dw1_t[:], dw1_r[:])
            eb1_t = persist.tile([P, H // P], F32, tag="eb1")
            eb2_t = persist.tile([P, H // P], F32, tag="eb2")
            eb3_t = persist.tile([P, L // P], F32, tag="eb3")
            db1_t = persist.tile([P, H // P], F32, tag="db1")
            db2_t = persist.tile([P, H // P], F32, tag="db2")
            db3_t = persist.tile([P, D // P], F32, tag="db3")
            for t, d in [(eb1_t, eb1_d), (eb2_t, eb2_d), (eb3_t, eb3_d),
                         (db1_t, db1_d), (db2_t, db2_d), (db3_t, db3_d)]:
                nc.sync.dma_start(t[:], d.ap())
            ident_t = persist.tile([P, P], F32, tag="ident")
            nc.sync.dma_start(ident_t[:], ident_d.ap())
            ones1_t = persist.tile([1, P], F32R, tag="ones1")
            nc.sync.dma_start(ones1_t[:], ones1_d.ap())
            ones128_t = persist.tile([P, 1], F32R, tag="ones128")
            nc.sync.dma_start(ones128_t[:], ones128_d.ap())
            mch_hi = persist.tile([1, K], F32R, tag="mch_hi")
            mch_lo = persist.tile([1, K], F32R, tag="mch_lo")
            zeT = persist.tile([P, L // P, NS], F32R, tag="zeT")    # 2 MB
            zqT = persist.tile([P, L // P, NS], F32R, tag="zqT")    # 2 MB
            s1buf = persist.tile([P, NST], F32, tag="s1buf")
            s2buf = persist.tile([P, NTILE], F32, tag="s2buf")
            nc.vector.memset(s1buf[:], 0.0)
            nc.vector.memset(s2buf[:], 0.0)

            if repeat > 1:
                stack.enter_context(tc.For_i(0, repeat, 1))

            # ================= P1: encoder =================
            if 1 in phases:
              with tc.tile_pool(name="encw", bufs=1) as encw, \
                 tc.tile_pool(name="encwork", bufs=2) as work:
                ew1_t = encw.tile([P, D // P, H], F32R, tag="ew1")
                ew2_t = encw.tile([P, H // P, H], F32R, tag="ew2")
                ew3_t = encw.tile([P, H // P, L], F32R, tag="ew3")
                nc.sync.dma_start(ew1_t[:], ew1_r[:])
                nc.sync.dma_start(ew2_t[:], ew2_r[:])
                nc.sync.dma_start(ew3_t[:], ew3_r[:])

                for st in range(NST):
                    rs = st * RBLK
                    xt = work.tile([P, D // P, RBLK], F32R, tag="xt")
                    nc.sync.dma_start(xt[:], xT_r[st].bitcast(F32R))
                    h1 = work.tile([P, H // P, RBLK], F32R, tag="h1")
                    for f in range(H // P):
                        pt = ps_mm.tile([P, RBLK], F32, tag="ps_enc")
                        for d_ in range(D // P):
                            nc.tensor.matmul(
                                pt[:], ew1_t[:, d_, f * P:(f + 1) * P], xt[:, d_, :],
                                start=(d_ == 0), stop=(d_ == D // P - 1),
                            )
                        nc.scalar.activation(h1[:, f, :], pt[:], AF.Gelu,
                                             bias=eb1_t[:, f:f + 1])
                    h2 = work.tile([P, H // P, RBLK], F32R, tag="h2")
                    for f in range(H // P):
                        pt = ps_mm.tile([P, RBLK], F32, tag="ps_enc")
                        for d_ in range(H // P):
                            nc.tensor.matmul(
                                pt[:], ew2_t[:, d_, f * P:(f + 1) * P], h1[:, d_, :],
                                start=(d_ == 0), stop=(d_ == H // P - 1),
                            )
                        nc.scalar.activation(h2[:, f, :], pt[:], AF.Gelu,
                                             bias=eb2_t[:, f:f + 1])
                    for f in range(L // P):
                        pt = ps_mm.tile([P, RBLK], F32, tag="ps_enc")
                        for d_ in range(H // P):
                            nc.tensor.matmul(
                                pt[:], ew3_t[:, d_, f * P:(f + 1) * P], h2[:, d_, :],
                                start=(d_ == 0), stop=(d_ == H // P - 1),
                            )
                        nc.scalar.activation(zeT[:, f, rs:rs + RBLK], pt[:],
                                             AF.Identity, bias=eb3_t[:, f:f + 1])

            # decoder weights prefetched during P2 (right-side pool entered
            # after the embedT pool frees, registered on the outer stack so
            # the tiles survive into P3)
            dec_w = []

            def load_dec_weights():
                decw = stack.enter_context(
                    tc.tile_pool(name="decw", bufs=1, side="right"))
                dw2_t = decw.tile([P, H // P, H], F32R, tag="dw2")
                dw3_t = decw.tile([P, H // P, D], F32R, tag="dw3")
                for o in range(H // P):
                    nc.sync.dma_start(dw2_t[:, o], dw2_r[:, o])
                    nc.sync.dma_start(dw3_t[:, o], dw3_r[:, o])
                dec_w.append((dw2_t, dw3_t))

            # ================= P2: VQ =================
            if 2 in phases:
              with tc.tile_pool(name="vq", bufs=1) as vq, \
                 tc.tile_pool(name="vqwork", bufs=2) as vwork:
                p4096_t = vq.tile([P, 1], F32, tag="p4096")
                nc.sync.dma_start(p4096_t[:], p4096_d.ap())
                ones_i32 = vq.tile([P, 1], mybir.dt.int32, tag="ones_i32")
                nc.vector.memset(ones_i32[:], 1)
                ixu_all = vq.tile([P, NTILE], mybir.dt.uint32, tag="ixu_all")
                zq_all = vq.tile([P, NTILE, L], F32, tag="zq_all")
                oh_flat = oh_d.ap().rearrange("a b -> (a b)").unsqueeze(1)
                mch_f = vq.tile([1, K], F32, tag="mch_f")

                etp_cm = tc.tile_pool(name="etp", bufs=1)
                etp = etp_cm.__enter__()
                sqp_cm = tc.tile_pool(name="sqpool", bufs=2)
                sqp = sqp_cm.__enter__()
                et_t = etp.tile([P, L // P, K], F32R, tag="et")
                for lo in range(L // P):
                    nc.sync.dma_start(et_t[:, lo], eT_r[:, lo])

                # c build: mch = -||e_k||^2 / 2, split hi+lo in f32r
                for kc in range(NKC):
                    ks = kc * KC
                    sq = sqp.tile([P, L // P, KC], F32R, tag="sq")
                    for lo in range(L // P):
                        nc.scalar.activation(sq[:, lo, :], et_t[:, lo, ks:ks + KC],
                                             AF.Square)
                    cps = ps_big.tile([P, KC], F32, tag="ps_dist")
                    for lo in range(L // P):
                        nc.tensor.matmul(cps[0:1, :], ones128_t[:], sq[:, lo, :],
                                         start=(lo == 0), stop=(lo == L // P - 1))
                    nc.scalar.activation(mch_f[:, ks:ks + KC], cps[0:1, :],
                                         AF.Copy, scale=-0.5)
                nc.vector.tensor_copy(mch_hi[:], mch_f[:])
                # lo = mch_f - mch_hi (bitcast hi to f32 for the subtract)
                nc.vector.tensor_sub(mch_f[:], mch_f[:], mch_hi[:].bitcast(F32))
                nc.vector.tensor_copy(mch_lo[:], mch_f[:])

                for i in range(NTILE):
                    ri = i * P
                    sp = vwork.tile([P, K], F32, tag="sp")
                    for kc in range(NKC):
                        ks = kc * KC
                        dps = ps_big.tile([P, KC], F32, tag="ps_dist")
                        nc.tensor.matmul(dps[:], zeT[:, 0, ri:ri + P],
                                         et_t[:, 0, ks:ks + KC], start=True, stop=False)
                        nc.tensor.matmul(dps[:], zeT[:, 1, ri:ri + P],
                                         et_t[:, 1, ks:ks + KC], start=False, stop=False)
                        nc.tensor.matmul(dps[:], ones1_t[:], mch_hi[:, ks:ks + KC],
                                         start=False, stop=False)
                        nc.tensor.matmul(dps[:], ones1_t[:], mch_lo[:, ks:ks + KC],
                                         start=False, stop=True)
                        nc.scalar.copy(sp[:, ks:ks + KC], dps[:])
                    if p2_level < 2:
                        continue
                    mx8 = vwork.tile([P, 8], F32, tag="mx8")
                    ix8 = vwork.tile([P, 8], mybir.dt.uint32, tag="ix8")
                    nc.vector.max(mx8[:], sp[:])
                    nc.vector.max_index(ix8[:], mx8[:], sp[:])
                    ixf = vwork.tile([P, 1], F32, tag="ixf")
                    nc.vector.tensor_copy(ixf[:], ix8[:, 0:1])
                    nc.vector.tensor_copy(ixu_all[:, i:i + 1], ix8[:, 0:1])
                    if p2_level >= 3:
                        # z_discrete: scatter 128 int32 ones at flat offsets
                        # (output DRAM is zero-initialized by the runtime)
                        off_f = vwork.tile([P, 1], F32, tag="off_f")
                        nc.vector.tensor_scalar(off_f[:], ixf[:], p4096_t[:],
                                                float(ri * K), ALU.add, ALU.add)
                        off_i = vwork.tile([P, 1], mybir.dt.int32, tag="off_i")
                        nc.vector.tensor_copy(off_i[:], off_f[:])
                        nc.gpsimd.indirect_dma_start(
                            out=oh_flat, out_offset=IndirectOffsetOnAxis(
                                ap=off_i[:], axis=0),
                            in_=ones_i32[:], in_offset=None,
                        )
                    if p2_level >= 4:
                        # gather z_q rows from HBM
                        nc.gpsimd.indirect_dma_start(
                            out=zq_all[:, i, :], out_offset=None, in_=ep_d.ap(),
                            in_offset=IndirectOffsetOnAxis(
                                ap=ixu_all[:, i:i + 1], axis=0),
                        )
                sqp_cm.__exit__(None, None, None)
                etp_cm.__exit__(None, None, None)
                if 3 in phases:
                    load_dec_weights()
                if p2_level >= 5:
                    for i in range(NTILE):
                        ri = i * P
                        # transpose to feature-major (f32r for the decoder)
                        for lo in range(L // P):
                            tps = ps_tp.tile([P, P], F32, tag="tp")
                            nc.tensor.transpose(
                                tps[:], zq_all[:, i, lo * P:(lo + 1) * P],
                                ident_t[:])
                            nc.scalar.copy(zqT[:, lo, ri:ri + P], tps[:])
                        # codebook-loss partial: sum((z_e - z_q)^2)
                        df = vwork.tile([P, L // P, P], F32, tag="df")
                        nc.vector.tensor_sub(
                            df[:], zeT[:, :, ri:ri + P].bitcast(F32),
                            zqT[:, :, ri:ri + P].bitcast(F32))
                        nc.scalar.activation(df[:], df[:], AF.Square,
                                             accum_out=s2buf[:, i:i + 1])

            # ================= P3: decoder =================
            if 3 in phases:
              with tc.tile_pool(name="decwork", bufs=2) as dwork:
                if not dec_w:
                    load_dec_weights()
                dw2_t, dw3_t = dec_w[0]

                # supertiles processed in pairs: gelu layers for both, then
                # sigmoid layers for both (halves ACT table-set switches)
                for pr in range(NST // 2):
                    g2s = []
                    for st in (2 * pr, 2 * pr + 1):
                        rs = st * RBLK
                        g1 = dwork.tile([P, H // P, RBLK], F32R, tag="g1")
                        for f in range(H // P):
                            pt = ps_mm.tile([P, RBLK], F32, tag="ps_enc")
                            for d_ in range(L // P):
                                nc.tensor.matmul(
                                    pt[:], dw1_t[:, d_, f * P:(f + 1) * P],
                                    zqT[:, d_, rs:rs + RBLK],
                                    start=(d_ == 0), stop=(d_ == L // P - 1),
                                )
                            nc.scalar.activation(g1[:, f, :], pt[:], AF.Gelu,
                                                 bias=db1_t[:, f:f + 1])
                        g2 = dwork.tile([P, H // P, RBLK], F32R, tag="g2")
                        for f in range(H // P):
                            pt = ps_mm.tile([P, RBLK], F32, tag="ps_enc")
                            for d_ in range(H // P):
                                nc.tensor.matmul(
                                    pt[:], dw2_t[:, d_, f * P:(f + 1) * P],
                                    g1[:, d_, :],
                                    start=(d_ == 0), stop=(d_ == H // P - 1),
                                )
                            nc.scalar.activation(g2[:, f, :], pt[:], AF.Gelu,
                                                 bias=db2_t[:, f:f + 1])
                        g2s.append((st, g2))
                    for st, g2 in g2s:
                        xp = dwork.tile([P, D // P, RBLK], F32, tag="xp")
                        for f in range(D // P):
                            pt = ps_mm.tile([P, RBLK], F32, tag="ps_enc")
                            for d_ in range(H // P):
                                nc.tensor.matmul(
                                    pt[:], dw3_t[:, d_, f * P:(f + 1) * P],
                                    g2[:, d_, :],
                                    start=(d_ == 0), stop=(d_ == H // P - 1),
                                )
                            nc.scalar.activation(xp[:, f, :], pt[:], AF.Sigmoid,
                                                 bias=db3_t[:, f:f + 1])
                        nc.sync.dma_start(xpT_r[st], xp[:])
                        # recon-loss partial: sum((x - x_pred)^2)
                        xtf = dwork.tile([P, D // P, RBLK], F32, tag="xtf")
                        nc.sync.dma_start(xtf[:], xT_r[st])
                        nc.vector.tensor_sub(xtf[:], xtf[:], xp[:])
                        nc.scalar.activation(xtf[:], xtf[:], AF.Square,
                                             accum_out=s1buf[:, st:st + 1])

            # ================= P4: loss partials out =================
            lp = persist.tile([P, 2], F32, tag="lossp")
            nc.vector.reduce_sum(lp[:, 0:1], s1buf[:], axis=mybir.AxisListType.X)
            nc.vector.reduce_sum(lp[:, 1:2], s2buf[:], axis=mybir.AxisListType.X)
            nc.sync.dma_start(lossp_d.ap(), lp[:])

    nc.finalize()
    return nc


def _wtile(w):
    """[K_in, F] -> [128, K_in//128, F] (partition-major chunks, contiguous
    per partition)."""
    w = np.asarray(w, np.float32)
    return np.ascontiguousarray(w.reshape(-1, P, w.shape[1]).transpose(1, 0, 2))


def _prep_shared(inputs):
    ep = np.ascontiguousarray(inputs["embed_pool"], dtype=np.float32)
    shared = {
        "eT": _wtile(np.ascontiguousarray(ep.T)),
        "ep": ep,
        "ew1": _wtile(inputs["ew1"]),
        "ew2": _wtile(inputs["ew2"]),
        "ew3": _wtile(inputs["ew3"]),
        "dw1": _wtile(inputs["dw1"]),
        "dw2": _wtile(inputs["dw2"]),
        "dw3": _wtile(inputs["dw3"]),
        "eb1": np.ascontiguousarray(np.asarray(inputs["eb1"], np.float32).reshape(-1, P).T),
        "eb2": np.ascontiguousarray(np.asarray(inputs["eb2"], np.float32).reshape(-1, P).T),
        "eb3": np.ascontiguousarray(np.asarray(inputs["eb3"], np.float32).reshape(-1, P).T),
        "db1": np.ascontiguousarray(np.asarray(inputs["db1"], np.float32).reshape(-1, P).T),
        "db2": np.ascontiguousarray(np.asarray(inputs["db2"], np.float32).reshape(-1, P).T),
        "db3": np.ascontiguousarray(np.asarray(inputs["db3"], np.float32).reshape(-1, P).T),
        "p4096": (np.arange(P, dtype=np.float32) * K).reshape(P, 1),
        "ident": np.eye(P, dtype=np.float32),
        "ones1": np.ones((1, P), np.float32),
        "ones128": np.ones((P, 1), np.float32),
    }
    return shared


def _run(inputs, trace=False):
    if "nc" not in _cache:
        _cache["nc"] = _build()
    nc = _cache["nc"]
    return _run_nc(nc, inputs, trace)


def _run_nc(nc, inputs, trace=False):
    in_maps = _make_in_maps(inputs)
    res = run_bass_kernel_spmd(nc, in_maps, core_ids=list(range(NCORES)),
                               trace=trace)
    return _assemble(res.results), res


def _assemble(results):
    x_pred = np.empty((N, D), np.float32)
    z_disc = np.empty((N, K), np.int32)
    s1 = 0.0
    s2 = 0.0
    for c, r in enumerate(results):
        x_pred[c * NS:(c + 1) * NS] = _xuntile(r["xpredT"])
        z_disc[c * NS:(c + 1) * NS] = r["onehot"]
        s1 += r["lossp"][:, 0].astype(np.float64).sum()
        s2 += r["lossp"][:, 1].astype(np.float64).sum()
    loss = np.float32((s1 + 1.25 * s2) / N)
    return (x_pred, z_disc, loss)


def kernel(**inputs):
    out, _ = _run(inputs, trace=False)
    return out


def _bench_nc(nc, in_maps, iters):
    """Build the sharded jit once for `nc`, keep inputs device-resident,
    re-donate outputs; return (times, host_outs_of_last_iter)."""
    import time

    import jax
    from jax.sharding import Mesh, NamedSharding, PartitionSpec
    from jax.experimental.shard_map import shard_map

    from concourse import bass2jax as B2J

    B2J.install_neuronx_cc_hook()
    partition_name = nc.partition_id_tensor.name if nc.partition_id_tensor else None
    in_names, out_names, out_avals, zero_outs = [], [], [], []
    for alloc in nc.m.functions[0].allocations:
        if not isinstance(alloc, mybir.MemoryLocationSet):
            continue
        name = alloc.memorylocations[0].name
        if alloc.kind == "ExternalInput":
            if name != partition_name:
                in_names.append(name)
        elif alloc.kind == "ExternalOutput":
            out_names.append(name)
            shape = tuple(alloc.tensor_shape)
            dtype = mybir.dt.np(alloc.dtype)
            out_avals.append(jax.core.ShapedArray(shape, dtype))
            zero_outs.append(np.zeros(shape, dtype))
    n_params = len(in_names)
    n_outs = len(out_avals)
    in_names_all = in_names + out_names + ([partition_name] if partition_name else [])
    donate = tuple(range(n_params, n_params + n_outs))

    def _body(*args):
        operands = list(args)
        if partition_name is not None:
            operands.append(B2J.partition_id_tensor())
        return tuple(B2J._bass_exec_p.bind(
            *operands, out_avals=tuple(out_avals), in_names=tuple(in_names_all),
            out_names=tuple(out_names), lowering_input_output_aliases=(),
            sim_require_finite=True, sim_require_nnan=True, nc=nc))

    devices = jax.devices()[:NCORES]
    mesh = Mesh(np.asarray(devices), ("core",))
    sharded = jax.jit(
        shard_map(_body, mesh=mesh,
                  in_specs=(PartitionSpec("core"),) * (n_params + n_outs),
                  out_specs=(PartitionSpec("core"),) * n_outs, check_rep=False),
        donate_argnums=donate, keep_unused=True)

    sh = NamedSharding(mesh, PartitionSpec("core"))
    concat_in = [
        jax.device_put(
            np.concatenate([np.asarray(in_maps[c][nm]) for c in range(NCORES)], 0), sh)
        for nm in in_names
    ]
    concat_zeros = [
        jax.device_put(np.zeros((NCORES * z.shape[0], *z.shape[1:]), z.dtype), sh)
        for z in zero_outs
    ]
    outs = sharded(*concat_in, *concat_zeros)
    jax.block_until_ready(outs)
    times = []
    for _ in range(iters):
        t0 = time.perf_counter()
        outs = sharded(*concat_in, *outs)
        jax.block_until_ready(outs)
        times.append(time.perf_counter() - t0)
    host_outs = [
        {nm: np.asarray(outs[i]).reshape(NCORES, *out_avals[i].shape)[c]
         for i, nm in enumerate(out_names)}
        for c in range(NCORES)
    ]
    return times, host_outs


def _xtile(x_shard):
    """[NS, D] -> [NST, 128, D//128, RBLK]: xtile[st, p, o, r] =
    x[st*RBLK + r, o*128 + p]."""
    v = x_shard.reshape(NST, RBLK, D // P, P)
    return np.ascontiguousarray(v.transpose(0, 3, 2, 1))


def _xuntile(xt):
    """Inverse of _xtile: [NST, 128, D//128, RBLK] -> [NS, D]."""
    return np.ascontiguousarray(
        xt.transpose(0, 3, 2, 1).reshape(NS, D))


def _make_in_maps(inputs):
    x = np.ascontiguousarray(np.asarray(inputs["x"], np.float32))
    shared = _prep_shared(inputs)
    in_maps = []
    for c in range(NCORES):
        m = dict(shared)
        m["xT"] = _xtile(x[c * NS:(c + 1) * NS])
        in_maps.append(m)
    return in_maps


def bench(inputs, iters=8, repeat=65):
    """Two-point measurement: time repeat=1 and repeat=R NEFFs in the same
    session; device time = (min(tR) - min(t1)) / (R - 1). min is robust to
    the bimodal axon dispatch-overhead noise."""
    in_maps = _make_in_maps(inputs)
    if "nc" not in _cache:
        _cache["nc"] = _build()
    if ("ncR", repeat) not in _cache:
        _cache[("ncR", repeat)] = _build(repeat=repeat)
    t1, host_outs = _bench_nc(_cache["nc"], in_maps, iters)
    tR, host_outs_R = _bench_nc(_cache[("ncR", repeat)], in_maps, iters)
    dev = (min(tR) - min(t1)) / (repeat - 1)
    return dev, t1, tR, host_outs, host_outs_R


# revision 55
# speedup vs baseline: 2.9957x; 1.1232x over previous
"""MinVQVAE1D forward pass on 8 Trainium2 NeuronCores.

Data-parallel: batch N=16384 sharded 2048 rows/core; codebook + MLP weights
replicated. All matmuls run in float32r (fp32 storage, TF32-like PE mode at
bf16 speed). The VQ argmin is computed as argmax of (z_e . e_k - ||e_k||^2/2)
where the -c/2 term is folded into the PE accumulation as two K=1 ones-row
matmuls (hi+lo split so the c term keeps ~fp32 accuracy). Index extraction
uses the DVE max/max_index top-8 instructions; z_q rows come back via an
indirect-DMA gather from HBM. Loss partial sums are produced on-device and
finished on host.

Self-contained: hardcodes all shapes from the problem spec.
"""
import sys

sys.path.insert(0, "/opt/trn_rl_repo")

import numpy as np

import concourse.bass as bass
import concourse.mybir as mybir
import concourse.tile as tile
from concourse import bacc
from concourse.bass import IndirectOffsetOnAxis
from concourse.bass_utils import run_bass_kernel_spmd

# problem shapes
N, D, H, L, K = 16384, 1024, 1024, 256, 4096
NCORES = 8
NS = N // NCORES          # rows per core
P = 128
RBLK = 256                # supertile row block (moving free dim; >=256 keeps f32r at 1 cyc/row)
NST = NS // RBLK          # supertiles per core
NTILE = NS // P           # 128-row tiles per core (VQ phase)
KC = 512                  # distance k-chunk (one PSUM bank)
NKC = K // KC

F32 = mybir.dt.float32
F32R = mybir.dt.float32r
AF = mybir.ActivationFunctionType
ALU = mybir.AluOpType

_cache = {}


def _build(repeat=1, phases=(1, 2, 3), p2_level=5):
    nc = bacc.Bacc(None, target_bir_lowering=False)

    # ---- DRAM I/O ----
    # xT pre-tiled on host: [NST, 128, D//128, RBLK]; one contiguous-per-
    # partition DMA per supertile (128 descriptors x 8KB).
    xT_d = nc.dram_tensor("xT", [NST, P, D // P, RBLK], F32, kind="ExternalInput")
    eT_d = nc.dram_tensor("eT", [P, L // P, K], F32R, kind="ExternalInput")
    ep_d = nc.dram_tensor("ep", [K, L], F32, kind="ExternalInput")
    ew1_d = nc.dram_tensor("ew1", [P, D // P, H], F32R, kind="ExternalInput")
    ew2_d = nc.dram_tensor("ew2", [P, H // P, H], F32R, kind="ExternalInput")
    ew3_d = nc.dram_tensor("ew3", [P, H // P, L], F32R, kind="ExternalInput")
    dw1_d = nc.dram_tensor("dw1", [P, L // P, H], F32R, kind="ExternalInput")
    dw2_d = nc.dram_tensor("dw2", [P, H // P, H], F32R, kind="ExternalInput")
    dw3_d = nc.dram_tensor("dw3", [P, H // P, D], F32R, kind="ExternalInput")
    # biases pre-shaped [128, nchunks] on host
    eb1_d = nc.dram_tensor("eb1", [P, H // P], F32, kind="ExternalInput")
    eb2_d = nc.dram_tensor("eb2", [P, H // P], F32, kind="ExternalInput")
    eb3_d = nc.dram_tensor("eb3", [P, L // P], F32, kind="ExternalInput")
    db1_d = nc.dram_tensor("db1", [P, H // P], F32, kind="ExternalInput")
    db2_d = nc.dram_tensor("db2", [P, H // P], F32, kind="ExternalInput")
    db3_d = nc.dram_tensor("db3", [P, D // P], F32, kind="ExternalInput")
    p4096_d = nc.dram_tensor("p4096", [P, 1], F32, kind="ExternalInput")
    ident_d = nc.dram_tensor("ident", [P, P], F32, kind="ExternalInput")
    ones2_d = nc.dram_tensor("ones2", [2, P], F32R, kind="ExternalInput")
    ones128_d = nc.dram_tensor("ones128", [P, 1], F32R, kind="ExternalInput")

    xpT_d = nc.dram_tensor("xpredT", [NST, P, D // P, RBLK], F32, kind="ExternalOutput")
    oh_d = nc.dram_tensor("onehot", [NS, K], mybir.dt.int32, kind="ExternalOutput")
    lossp_d = nc.dram_tensor("lossp", [P, 2], F32, kind="ExternalOutput")

    xT_r = xT_d.ap()
    xpT_r = xpT_d.ap()
    ew1_r = ew1_d.ap()
    ew2_r = ew2_d.ap()
    ew3_r = ew3_d.ap()
    dw1_r = dw1_d.ap()
    dw2_r = dw2_d.ap()
    dw3_r = dw3_d.ap()
    eT_r = eT_d.ap()
    oh_r = oh_d.ap()

    with tile.TileContext(nc) as tc:
        import contextlib

        stack = contextlib.ExitStack()
        with stack:
            persist = stack.enter_context(tc.tile_pool(name="persist", bufs=1))
            ps_mm = stack.enter_context(tc.tile_pool(name="ps_mm", bufs=3, space="PSUM"))
            ps_big = stack.enter_context(tc.tile_pool(name="ps_big", bufs=4, space="PSUM"))
            ps_tp = stack.enter_context(tc.tile_pool(name="ps_tp", bufs=1, space="PSUM"))

            # ---- persistent small tensors ----
            dw1_t = persist.tile([P, L // P, H], F32R, tag="dw1")
            nc.sync.dma_start(dw1_t[:], dw1_r[:])
            eb1_t = persist.tile([P, H // P], F32, tag="eb1")
            eb2_t = persist.tile([P, H // P], F32, tag="eb2")
            eb3_t = persist.tile([P, L // P], F32, tag="eb3")
            db1_t = persist.tile([P, H // P], F32, tag="db1")
            db2_t = persist.tile([P, H // P], F32, tag="db2")
            db3_t = persist.tile([P, D // P], F32, tag="db3")
            for t, d in [(eb1_t, eb1_d), (eb2_t, eb2_d), (eb3_t, eb3_d),
                         (db1_t, db1_d), (db2_t, db2_d), (db3_t, db3_d)]:
                nc.sync.dma_start(t[:], d.ap())
            ident_t = persist.tile([P, P], F32, tag="ident")
            nc.sync.dma_start(ident_t[:], ident_d.ap())
            ones2_t = persist.tile([2, P], F32R, tag="ones2")
            nc.sync.dma_start(ones2_t[:], ones2_d.ap())
            ones128_t = persist.tile([P, 1], F32R, tag="ones128")
            nc.sync.dma_start(ones128_t[:], ones128_d.ap())
            # mch2[0] = f32r(-c/2), mch2[1] = residual; one K=2 matmul adds both
            mch2 = persist.tile([2, K], F32R, tag="mch2")
            zeT = persist.tile([P, L // P, NS], F32R, tag="zeT")    # 2 MB
            zqT = persist.tile([P, L // P, NS], F32R, tag="zqT")    # 2 MB
            s1buf = persist.tile([P, NST], F32, tag="s1buf")
            s2buf = persist.tile([P, NTILE], F32, tag="s2buf")
            nc.vector.memset(s1buf[:], 0.0)
            nc.vector.memset(s2buf[:], 0.0)

            if repeat > 1:
                stack.enter_context(tc.For_i(0, repeat, 1))

            # ================= P1: encoder =================
            if 1 in phases:
              with tc.tile_pool(name="encw", bufs=1) as encw, \
                 tc.tile_pool(name="encwork", bufs=2) as work:
                ew1_t = encw.tile([P, D // P, H], F32R, tag="ew1")
                ew2_t = encw.tile([P, H // P, H], F32R, tag="ew2")
                ew3_t = encw.tile([P, H // P, L], F32R, tag="ew3")
                nc.sync.dma_start(ew1_t[:], ew1_r[:])
                nc.sync.dma_start(ew2_t[:], ew2_r[:])
                nc.sync.dma_start(ew3_t[:], ew3_r[:])

                for st in range(NST):
                    rs = st * RBLK
                    xt = work.tile([P, D // P, RBLK], F32R, tag="xt")
                    nc.sync.dma_start(xt[:], xT_r[st].bitcast(F32R))
                    h1 = work.tile([P, H // P, RBLK], F32R, tag="h1")
                    for f in range(H // P):
                        pt = ps_mm.tile([P, RBLK], F32, tag="ps_enc")
                        for d_ in range(D // P):
                            nc.tensor.matmul(
                                pt[:], ew1_t[:, d_, f * P:(f + 1) * P], xt[:, d_, :],
                                start=(d_ == 0), stop=(d_ == D // P - 1),
                            )
                        nc.scalar.activation(h1[:, f, :], pt[:], AF.Gelu,
                                             bias=eb1_t[:, f:f + 1])
                    h2 = work.tile([P, H // P, RBLK], F32R, tag="h2")
                    for f in range(H // P):
                        pt = ps_mm.tile([P, RBLK], F32, tag="ps_enc")
                        for d_ in range(H // P):
                            nc.tensor.matmul(
                                pt[:], ew2_t[:, d_, f * P:(f + 1) * P], h1[:, d_, :],
                                start=(d_ == 0), stop=(d_ == H // P - 1),
                            )
                        nc.scalar.activation(h2[:, f, :], pt[:], AF.Gelu,
                                             bias=eb2_t[:, f:f + 1])
                    for f in range(L // P):
                        pt = ps_mm.tile([P, RBLK], F32, tag="ps_enc")
                        for d_ in range(H // P):
                            nc.tensor.matmul(
                                pt[:], ew3_t[:, d_, f * P:(f + 1) * P], h2[:, d_, :],
                                start=(d_ == 0), stop=(d_ == H // P - 1),
                            )
                        nc.scalar.activation(zeT[:, f, rs:rs + RBLK], pt[:],
                                             AF.Identity, bias=eb3_t[:, f:f + 1])

            # decoder weights prefetched during P2 (right-side pool entered
            # after the embedT pool frees, registered on the outer stack so
            # the tiles survive into P3)
            dec_w = []

            def load_dec_weights():
                decw = stack.enter_context(
                    tc.tile_pool(name="decw", bufs=1, side="right"))
                dw2_t = decw.tile([P, H // P, H], F32R, tag="dw2")
                dw3_t = decw.tile([P, H // P, D], F32R, tag="dw3")
                for o in range(H // P):
                    nc.sync.dma_start(dw2_t[:, o], dw2_r[:, o])
                    nc.sync.dma_start(dw3_t[:, o], dw3_r[:, o])
                dec_w.append((dw2_t, dw3_t))

            # ================= P2: VQ =================
            if 2 in phases:
              with tc.tile_pool(name="vq", bufs=1) as vq, \
                 tc.tile_pool(name="vqwork", bufs=2) as vwork:
                p4096_t = vq.tile([P, 1], F32, tag="p4096")
                nc.sync.dma_start(p4096_t[:], p4096_d.ap())
                ones_i32 = vq.tile([P, 1], mybir.dt.int32, tag="ones_i32")
                nc.vector.memset(ones_i32[:], 1)
                ixu_all = vq.tile([P, NTILE], mybir.dt.uint32, tag="ixu_all")
                zq_all = vq.tile([P, NTILE, L], F32, tag="zq_all")
                oh_flat = oh_d.ap().rearrange("a b -> (a b)").unsqueeze(1)

                spp_cm = tc.tile_pool(name="spp", bufs=3)
                spp = spp_cm.__enter__()
                etp_cm = tc.tile_pool(name="etp", bufs=1)
                etp = etp_cm.__enter__()
                sqp_cm = tc.tile_pool(name="sqpool", bufs=2)
                sqp = sqp_cm.__enter__()
                cbp_cm = tc.tile_pool(name="cbuild", bufs=1)
                cbp = cbp_cm.__enter__()
                mch_f = cbp.tile([1, K], F32, tag="mch_f")
                et_t = etp.tile([P, L // P, K], F32R, tag="et")
                for lo in range(L // P):
                    for kh in range(2):
                        nc.sync.dma_start(et_t[:, lo, kh * (K // 2):(kh + 1) * (K // 2)],
                                          eT_r[:, lo, kh * (K // 2):(kh + 1) * (K // 2)])

                # c build: mch = -||e_k||^2 / 2, split hi+lo in f32r
                for kc in range(NKC):
                    ks = kc * KC
                    sq = sqp.tile([P, L // P, KC], F32R, tag="sq")
                    for lo in range(L // P):
                        nc.scalar.activation(sq[:, lo, :], et_t[:, lo, ks:ks + KC],
                                             AF.Square)
                    cps = ps_big.tile([P, KC], F32, tag="ps_dist")
                    for lo in range(L // P):
                        nc.tensor.matmul(cps[0:1, :], ones128_t[:], sq[:, lo, :],
                                         start=(lo == 0), stop=(lo == L // P - 1))
                    nc.scalar.activation(mch_f[:, ks:ks + KC], cps[0:1, :],
                                         AF.Copy, scale=-0.5)
                nc.vector.tensor_copy(mch2[0:1, :], mch_f[:])
                # residual = mch_f - f32r(mch_f) (bitcast hi to f32 to subtract)
                nc.vector.tensor_sub(mch_f[:], mch_f[:], mch2[0:1, :].bitcast(F32))
                mch_res = cbp.tile([1, K], F32R, tag="mch_res")
                nc.vector.tensor_copy(mch_res[:], mch_f[:])
                # compute engines can't address base_partition=1; DMA can
                nc.sync.dma_start(mch2[1:2, :], mch_res[:])

                cbp_cm.__exit__(None, None, None)
                sqp_cm.__exit__(None, None, None)

                for i in range(NTILE):
                    ri = i * P
                    sp = spp.tile([P, K], F32, tag="sp")
                    for kc in range(NKC):
                        ks = kc * KC
                        dps = ps_big.tile([P, KC], F32, tag="ps_dist")
                        nc.tensor.matmul(dps[:], zeT[:, 0, ri:ri + P],
                                         et_t[:, 0, ks:ks + KC], start=True, stop=False)
                        nc.tensor.matmul(dps[:], zeT[:, 1, ri:ri + P],
                                         et_t[:, 1, ks:ks + KC], start=False, stop=False)
                        nc.tensor.matmul(dps[:], ones2_t[:], mch2[:, ks:ks + KC],
                                         start=False, stop=True)
                        nc.scalar.copy(sp[:, ks:ks + KC], dps[:])
                    if p2_level < 2:
                        continue
                    mx8 = vwork.tile([P, 8], F32, tag="mx8")
                    ix8 = vwork.tile([P, 8], mybir.dt.uint32, tag="ix8")
                    nc.vector.max(mx8[:], sp[:])
                    nc.vector.max_index(ix8[:], mx8[:], sp[:])
                    ixf = vwork.tile([P, 1], F32, tag="ixf")
                    nc.vector.tensor_copy(ixf[:], ix8[:, 0:1])
                    nc.vector.tensor_copy(ixu_all[:, i:i + 1], ix8[:, 0:1])
                    if p2_level >= 3:
                        # z_discrete: scatter 128 int32 ones at flat offsets
                        # (output DRAM is zero-initialized by the runtime)
                        off_f = vwork.tile([P, 1], F32, tag="off_f")
                        nc.vector.tensor_scalar(off_f[:], ixf[:], p4096_t[:],
                                                float(ri * K), ALU.add, ALU.add)
                        off_i = vwork.tile([P, 1], mybir.dt.int32, tag="off_i")
                        nc.vector.tensor_copy(off_i[:], off_f[:])
                        nc.gpsimd.indirect_dma_start(
                            out=oh_flat, out_offset=IndirectOffsetOnAxis(
                                ap=off_i[:], axis=0),
                            in_=ones_i32[:], in_offset=None,
                        )
                    if p2_level >= 4:
                        # gather z_q rows from HBM
                        nc.gpsimd.indirect_dma_start(
                            out=zq_all[:, i, :], out_offset=None, in_=ep_d.ap(),
                            in_offset=IndirectOffsetOnAxis(
                                ap=ixu_all[:, i:i + 1], axis=0),
                        )
                etp_cm.__exit__(None, None, None)
                spp_cm.__exit__(None, None, None)
                if 3 in phases:
                    load_dec_weights()
                if p2_level >= 5:
                    for i in range(NTILE):
                        ri = i * P
                        # transpose to feature-major (f32r for the decoder)
                        for lo in range(L // P):
                            tps = ps_tp.tile([P, P], F32, tag="tp")
                            nc.tensor.transpose(
                                tps[:], zq_all[:, i, lo * P:(lo + 1) * P],
                                ident_t[:])
                            nc.scalar.copy(zqT[:, lo, ri:ri + P], tps[:])
                        # codebook-loss partial: sum((z_e - z_q)^2)
                        df = vwork.tile([P, L // P, P], F32, tag="df")
                        nc.vector.tensor_sub(
                            df[:], zeT[:, :, ri:ri + P].bitcast(F32),
                            zqT[:, :, ri:ri + P].bitcast(F32))
                        nc.scalar.activation(df[:], df[:], AF.Square,
                                             accum_out=s2buf[:, i:i + 1])

            # ================= P3: decoder =================
            if 3 in phases:
              with tc.tile_pool(name="decwork", bufs=2) as dwork:
                if not dec_w:
                    load_dec_weights()
                dw2_t, dw3_t = dec_w[0]

                # supertiles processed in pairs: gelu layers for both, then
                # sigmoid layers for both (halves ACT table-set switches)
                for pr in range(NST // 2):
                    g2s = []
                    for st in (2 * pr, 2 * pr + 1):
                        rs = st * RBLK
                        g1 = dwork.tile([P, H // P, RBLK], F32R, tag="g1")
                        for f in range(H // P):
                            pt = ps_mm.tile([P, RBLK], F32, tag="ps_enc")
                            for d_ in range(L // P):
                                nc.tensor.matmul(
                                    pt[:], dw1_t[:, d_, f * P:(f + 1) * P],
                                    zqT[:, d_, rs:rs + RBLK],
                                    start=(d_ == 0), stop=(d_ == L // P - 1),
                                )
                            nc.scalar.activation(g1[:, f, :], pt[:], AF.Gelu,
                                                 bias=db1_t[:, f:f + 1])
                        g2 = dwork.tile([P, H // P, RBLK], F32R, tag="g2")
                        for f in range(H // P):
                            pt = ps_mm.tile([P, RBLK], F32, tag="ps_enc")
                            for d_ in range(H // P):
                                nc.tensor.matmul(
                                    pt[:], dw2_t[:, d_, f * P:(f + 1) * P],
                                    g1[:, d_, :],
                                    start=(d_ == 0), stop=(d_ == H // P - 1),
                                )
                            nc.scalar.activation(g2[:, f, :], pt[:], AF.Gelu,
                                                 bias=db2_t[:, f:f + 1])
                        g2s.append((st, g2))
                    for st, g2 in g2s:
                        xp = dwork.tile([P, D // P, RBLK], F32, tag="xp")
                        for f in range(D // P):
                            pt = ps_mm.tile([P, RBLK], F32, tag="ps_enc")
                            for d_ in range(H // P):
                                nc.tensor.matmul(
                                    pt[:], dw3_t[:, d_, f * P:(f + 1) * P],
                                    g2[:, d_, :],
                                    start=(d_ == 0), stop=(d_ == H // P - 1),
                                )
                            nc.scalar.activation(xp[:, f, :], pt[:], AF.Sigmoid,
                                                 bias=db3_t[:, f:f + 1])
                        nc.sync.dma_start(xpT_r[st], xp[:])
                        # recon-loss partial: sum((x - x_pred)^2)
                        xtf = dwork.tile([P, D // P, RBLK], F32, tag="xtf")
                        nc.sync.dma_start(xtf[:], xT_r[st])
                        nc.vector.tensor_sub(xtf[:], xtf[:], xp[:])
                        nc.scalar.activation(xtf[:], xtf[:], AF.Square,
                                             accum_out=s1buf[:, st:st + 1])

            # ================= P4: loss partials out =================
            lp = persist.tile([P, 2], F32, tag="lossp")
            nc.vector.reduce_sum(lp[:, 0:1], s1buf[:], axis=mybir.AxisListType.X)
            nc.vector.reduce_sum(lp[:, 1:2], s2buf[:], axis=mybir.AxisListType.X)
            nc.sync.dma_start(lossp_d.ap(), lp[:])

    nc.finalize()
    return nc


def _wtile(w):
    """[K_in, F] -> [128, K_in//128, F] (partition-major chunks, contiguous
    per partition)."""
    w = np.asarray(w, np.float32)
    return np.ascontiguousarray(w.reshape(-1, P, w.shape[1]).transpose(1, 0, 2))


def _prep_shared(inputs):
    ep = np.ascontiguousarray(inputs["embed_pool"], dtype=np.float32)
    shared = {
        "eT": _wtile(np.ascontiguousarray(ep.T)),
        "ep": ep,
        "ew1": _wtile(inputs["ew1"]),
        "ew2": _wtile(inputs["ew2"]),
        "ew3": _wtile(inputs["ew3"]),
        "dw1": _wtile(inputs["dw1"]),
        "dw2": _wtile(inputs["dw2"]),
        "dw3": _wtile(inputs["dw3"]),
        "eb1": np.ascontiguousarray(np.asarray(inputs["eb1"], np.float32).reshape(-1, P).T),
        "eb2": np.ascontiguousarray(np.asarray(inputs["eb2"], np.float32).reshape(-1, P).T),
        "eb3": np.ascontiguousarray(np.asarray(inputs["eb3"], np.float32).reshape(-1, P).T),
        "db1": np.ascontiguousarray(np.asarray(inputs["db1"], np.float32).reshape(-1, P).T),
        "db2": np.ascontiguousarray(np.asarray(inputs["db2"], np.float32).reshape(-1, P).T),
        "db3": np.ascontiguousarray(np.asarray(inputs["db3"], np.float32).reshape(-1, P).T),
        "p4096": (np.arange(P, dtype=np.float32) * K).reshape(P, 1),
        "ident": np.eye(P, dtype=np.float32),
        "ones2": np.ones((2, P), np.float32),
        "ones128": np.ones((P, 1), np.float32),
    }
    return shared


def _run(inputs, trace=False):
    if "nc" not in _cache:
        _cache["nc"] = _build()
    nc = _cache["nc"]
    return _run_nc(nc, inputs, trace)


def _run_nc(nc, inputs, trace=False):
    in_maps = _make_in_maps(inputs)
    res = run_bass_kernel_spmd(nc, in_maps, core_ids=list(range(NCORES)),
                               trace=trace)
    return _assemble(res.results), res


def _assemble(results):
    x_pred = np.empty((N, D), np.float32)
    z_disc = np.empty((N, K), np.int32)
    s1 = 0.0
    s2 = 0.0
    for c, r in enumerate(results):
        x_pred[c * NS:(c + 1) * NS] = _xuntile(r["xpredT"])
        z_disc[c * NS:(c + 1) * NS] = r["onehot"]
        s1 += r["lossp"][:, 0].astype(np.float64).sum()
        s2 += r["lossp"][:, 1].astype(np.float64).sum()
    loss = np.float32((s1 + 1.25 * s2) / N)
    return (x_pred, z_disc, loss)


def kernel(**inputs):
    out, _ = _run(inputs, trace=False)
    return out


def _bench_nc(nc, in_maps, iters):
    """Build the sharded jit once for `nc`, keep inputs device-resident,
    re-donate outputs; return (times, host_outs_of_last_iter)."""
    import time

    import jax
    from jax.sharding import Mesh, NamedSharding, PartitionSpec
    from jax.experimental.shard_map import shard_map

    from concourse import bass2jax as B2J

    B2J.install_neuronx_cc_hook()
    partition_name = nc.partition_id_tensor.name if nc.partition_id_tensor else None
    in_names, out_names, out_avals, zero_outs = [], [], [], []
    for alloc in nc.m.functions[0].allocations:
        if not isinstance(alloc, mybir.MemoryLocationSet):
            continue
        name = alloc.memorylocations[0].name
        if alloc.kind == "ExternalInput":
            if name != partition_name:
                in_names.append(name)
        elif alloc.kind == "ExternalOutput":
            out_names.append(name)
            shape = tuple(alloc.tensor_shape)
            dtype = mybir.dt.np(alloc.dtype)
            out_avals.append(jax.core.ShapedArray(shape, dtype))
            zero_outs.append(np.zeros(shape, dtype))
    n_params = len(in_names)
    n_outs = len(out_avals)
    in_names_all = in_names + out_names + ([partition_name] if partition_name else [])
    donate = tuple(range(n_params, n_params + n_outs))

    def _body(*args):
        operands = list(args)
        if partition_name is not None:
            operands.append(B2J.partition_id_tensor())
        return tuple(B2J._bass_exec_p.bind(
            *operands, out_avals=tuple(out_avals), in_names=tuple(in_names_all),
            out_names=tuple(out_names), lowering_input_output_aliases=(),
            sim_require_finite=True, sim_require_nnan=True, nc=nc))

    devices = jax.devices()[:NCORES]
    mesh = Mesh(np.asarray(devices), ("core",))
    sharded = jax.jit(
        shard_map(_body, mesh=mesh,
                  in_specs=(PartitionSpec("core"),) * (n_params + n_outs),
                  out_specs=(PartitionSpec("core"),) * n_outs, check_rep=False),
        donate_argnums=donate, keep_unused=True)

    sh = NamedSharding(mesh, PartitionSpec("core"))
    concat_in = [
        jax.device_put(
            np.concatenate([np.asarray(in_maps[c][nm]) for c in range(NCORES)], 0), sh)
        for nm in in_names
    ]
    concat_zeros = [
        jax.device_put(np.zeros((NCORES * z.shape[0], *z.shape[1:]), z.dtype), sh)
        for z in zero_outs
    ]
    outs = sharded(*concat_in, *concat_zeros)
    jax.block_until_ready(outs)
    times = []
    for _ in range(iters):
        t0 = time.perf_counter()
        outs = sharded(*concat_in, *outs)
        jax.block_until_ready(outs)
        times.append(time.perf_counter() - t0)
    host_outs = [
        {nm: np.asarray(outs[i]).reshape(NCORES, *out_avals[i].shape)[c]
         for i, nm in enumerate(out_names)}
        for c in range(NCORES)
    ]
    return times, host_outs


def _xtile(x_shard):
    """[NS, D] -> [NST, 128, D//128, RBLK]: xtile[st, p, o, r] =
    x[st*RBLK + r, o*128 + p]."""
    v = x_shard.reshape(NST, RBLK, D // P, P)
    return np.ascontiguousarray(v.transpose(0, 3, 2, 1))


def _xuntile(xt):
    """Inverse of _xtile: [NST, 128, D//128, RBLK] -> [NS, D]."""
    return np.ascontiguousarray(
        xt.transpose(0, 3, 2, 1).reshape(NS, D))


def _make_in_maps(inputs):
    x = np.ascontiguousarray(np.asarray(inputs["x"], np.float32))
    shared = _prep_shared(inputs)
    in_maps = []
    for c in range(NCORES):
        m = dict(shared)
        m["xT"] = _xtile(x[c * NS:(c + 1) * NS])
        in_maps.append(m)
    return in_maps


def bench(inputs, iters=8, repeat=65):
    """Two-point measurement: time repeat=1 and repeat=R NEFFs in the same
    session; device time = (min(tR) - min(t1)) / (R - 1). min is robust to
    the bimodal axon dispatch-overhead noise."""
    in_maps = _make_in_maps(inputs)
    if "nc" not in _cache:
        _cache["nc"] = _build()
    if ("ncR", repeat) not in _cache:
        _cache[("ncR", repeat)] = _build(repeat=repeat)
    t1, host_outs = _bench_nc(_cache["nc"], in_maps, iters)
    tR, host_outs_R = _bench_nc(_cache[("ncR", repeat)], in_maps, iters)
    dev = (min(tR) - min(t1)) / (repeat - 1)
    return dev, t1, tR, host_outs, host_outs_R


# revision 59
# speedup vs baseline: 3.2771x; 1.0939x over previous
"""MinVQVAE1D forward pass on 8 Trainium2 NeuronCores.

Data-parallel: batch N=16384 sharded 2048 rows/core; codebook + MLP weights
replicated. Encoder runs in bf16 (validated: zero argmin flips vs the fp32
reference on this problem's data distribution; min realized argmin gap is
~30x the worst-case bf16-encoder perturbation of the distance rows). The
distance matmul and decoder run in float32r (fp32 storage, TF32-like PE mode
at full speed). z_e itself is kept in f32r precision (fp32 PSUM of the bf16
matmul).

VQ argmin: s' = z_e . e_k - ||e_k||^2/2, with the -c/2 term folded into the
PE accumulation as one K=2 ones-row matmul (hi + residual rows so c keeps
~fp32 accuracy). Argmax via DVE max/max_index on two half-rows, combined
with a select. z_discrete is written as an indirect-DMA scatter of int32
ones (output DRAM is zero-initialized by the runtime; idempotent across
repeats). z_q rows come back via indirect-DMA gather from HBM.

The VQ work for each 256-row supertile is fused directly after its encoder
matmuls, so the DVE/ACT argmin work hides under encoder PE time.

Self-contained: hardcodes all shapes from the problem spec.
"""
import sys

sys.path.insert(0, "/opt/trn_rl_repo")

import contextlib

import numpy as np

import concourse.bass as bass
import concourse.mybir as mybir
import concourse.tile as tile
from concourse import bacc
from concourse.bass import IndirectOffsetOnAxis
from concourse.bass_utils import run_bass_kernel_spmd

# problem shapes
N, D, H, L, K = 16384, 1024, 1024, 256, 4096
NCORES = 8
NS = N // NCORES          # rows per core
P = 128
RBLK = 256                # supertile row block
NST = NS // RBLK          # supertiles per core
NTILE = NS // P           # 128-row tiles per core
KC = 512                  # distance k-chunk (one PSUM bank)
NKC = K // KC
KH = K // 2               # half-row for the two-phase argmax

F32 = mybir.dt.float32
F32R = mybir.dt.float32r
BF16 = mybir.dt.bfloat16
I32 = mybir.dt.int32
U32 = mybir.dt.uint32
AF = mybir.ActivationFunctionType
ALU = mybir.AluOpType

_cache = {}


def _build(repeat=1, phases=(1, 2, 3), p2_level=5):
    nc = bacc.Bacc(None, target_bir_lowering=False)

    # ---- DRAM I/O (host-pre-tiled layouts: contiguous per partition) ----
    xT_d = nc.dram_tensor("xT", [NST, P, D // P, RBLK], F32, kind="ExternalInput")
    xTb_d = nc.dram_tensor("xTb", [NST, P, D // P, RBLK], BF16, kind="ExternalInput")
    eT_d = nc.dram_tensor("eT", [P, L // P, K], F32R, kind="ExternalInput")
    ep_d = nc.dram_tensor("ep", [K, L], F32, kind="ExternalInput")
    ew1_d = nc.dram_tensor("ew1", [P, D // P, H], BF16, kind="ExternalInput")
    ew2_d = nc.dram_tensor("ew2", [P, H // P, H], BF16, kind="ExternalInput")
    ew3_d = nc.dram_tensor("ew3", [P, H // P, L], BF16, kind="ExternalInput")
    dw1_d = nc.dram_tensor("dw1", [P, L // P, H], F32R, kind="ExternalInput")
    dw2_d = nc.dram_tensor("dw2", [P, H // P, H], F32R, kind="ExternalInput")
    dw3_d = nc.dram_tensor("dw3", [P, H // P, D], F32R, kind="ExternalInput")
    eb1_d = nc.dram_tensor("eb1", [P, H // P], F32, kind="ExternalInput")
    eb2_d = nc.dram_tensor("eb2", [P, H // P], F32, kind="ExternalInput")
    eb3_d = nc.dram_tensor("eb3", [P, L // P], F32, kind="ExternalInput")
    db1_d = nc.dram_tensor("db1", [P, H // P], F32, kind="ExternalInput")
    db2_d = nc.dram_tensor("db2", [P, H // P], F32, kind="ExternalInput")
    db3_d = nc.dram_tensor("db3", [P, D // P], F32, kind="ExternalInput")
    p4096_d = nc.dram_tensor("p4096", [P, 1], F32, kind="ExternalInput")
    ident_d = nc.dram_tensor("ident", [P, P], F32, kind="ExternalInput")
    ones2_d = nc.dram_tensor("ones2", [2, P], F32R, kind="ExternalInput")
    ones128_d = nc.dram_tensor("ones128", [P, 1], F32R, kind="ExternalInput")

    xpT_d = nc.dram_tensor("xpredT", [NST, P, D // P, RBLK], F32, kind="ExternalOutput")
    oh_d = nc.dram_tensor("onehot", [NS, K], I32, kind="ExternalOutput")
    lossp_d = nc.dram_tensor("lossp", [P, 2], F32, kind="ExternalOutput")

    xT_r = xT_d.ap()
    xTb_r = xTb_d.ap()
    xpT_r = xpT_d.ap()
    eT_r = eT_d.ap()

    with tile.TileContext(nc) as tc:
        stack = contextlib.ExitStack()
        with stack:
            persist = stack.enter_context(tc.tile_pool(name="persist", bufs=1))
            ps_mm = stack.enter_context(tc.tile_pool(name="ps_mm", bufs=3, space="PSUM"))
            ps_big = stack.enter_context(tc.tile_pool(name="ps_big", bufs=4, space="PSUM"))
            ps_tp = stack.enter_context(tc.tile_pool(name="ps_tp", bufs=1, space="PSUM"))

            # ---- persistent small tensors ----
            dw1_t = persist.tile([P, L // P, H], F32R, tag="dw1")
            nc.sync.dma_start(dw1_t[:], dw1_d.ap())
            eb1_t = persist.tile([P, H // P], F32, tag="eb1")
            eb2_t = persist.tile([P, H // P], F32, tag="eb2")
            eb3_t = persist.tile([P, L // P], F32, tag="eb3")
            db1_t = persist.tile([P, H // P], F32, tag="db1")
            db2_t = persist.tile([P, H // P], F32, tag="db2")
            db3_t = persist.tile([P, D // P], F32, tag="db3")
            for t, d in [(eb1_t, eb1_d), (eb2_t, eb2_d), (eb3_t, eb3_d),
                         (db1_t, db1_d), (db2_t, db2_d), (db3_t, db3_d)]:
                nc.sync.dma_start(t[:], d.ap())
            ident_t = persist.tile([P, P], F32, tag="ident")
            nc.sync.dma_start(ident_t[:], ident_d.ap())
            ones2_t = persist.tile([2, P], F32R, tag="ones2")
            nc.sync.dma_start(ones2_t[:], ones2_d.ap())
            ones128_t = persist.tile([P, 1], F32R, tag="ones128")
            nc.sync.dma_start(ones128_t[:], ones128_d.ap())
            p4096_t = persist.tile([P, 1], F32, tag="p4096")
            nc.sync.dma_start(p4096_t[:], p4096_d.ap())
            ones_i32 = persist.tile([P, 1], I32, tag="ones_i32")
            nc.vector.memset(ones_i32[:], 1)
            zeT = persist.tile([P, L // P, NS], F32R, tag="zeT")      # 2 MB
            s1buf = persist.tile([P, NST], F32, tag="s1buf")
            s2buf = persist.tile([P, NTILE], F32, tag="s2buf")
            nc.vector.memset(s1buf[:], 0.0)
            nc.vector.memset(s2buf[:], 0.0)

            oh_flat = oh_d.ap().rearrange("a b -> (a b)").unsqueeze(1)

            # embedT: right-side pool, loaded from the very start
            et_cm = tc.tile_pool(name="etp", bufs=1, side="right")
            etp = et_cm.__enter__()
            et_t = etp.tile([P, L // P, K], F32R, tag="et")
            for lo in range(L // P):
                for kh in range(2):
                    nc.sync.dma_start(et_t[:, lo, kh * KH:(kh + 1) * KH],
                                      eT_r[:, lo, kh * KH:(kh + 1) * KH])

            if repeat > 1:
                stack.enter_context(tc.For_i(0, repeat, 1))

            vq_cm = tc.tile_pool(name="vq", bufs=1)
            vq = vq_cm.__enter__()
            vwork_cm = tc.tile_pool(name="vqwork", bufs=2)
            vwork = vwork_cm.__enter__()
            mch2 = vq.tile([2, K], F32R, tag="mch2")
            zq_all = vq.tile([P, NTILE, L], F32, tag="zq_all")

            # ---- c build: mch = -||e_k||^2/2 (hi + residual rows) ----
            if 2 in phases:
                with tc.tile_pool(name="cbuild", bufs=1) as cbp, \
                     tc.tile_pool(name="sqpool", bufs=2) as sqp:
                    mch_f = cbp.tile([1, K], F32, tag="mch_f")
                    mch_res = cbp.tile([1, K], F32R, tag="mch_res")
                    for kc in range(NKC):
                        ks = kc * KC
                        sq = sqp.tile([P, L // P, KC], F32R, tag="sq")
                        for lo in range(L // P):
                            nc.scalar.activation(sq[:, lo, :],
                                                 et_t[:, lo, ks:ks + KC], AF.Square)
                        cps = ps_big.tile([P, KC], F32, tag="ps_dist")
                        for lo in range(L // P):
                            nc.tensor.matmul(cps[0:1, :], ones128_t[:], sq[:, lo, :],
                                             start=(lo == 0), stop=(lo == L // P - 1))
                        nc.scalar.activation(mch_f[:, ks:ks + KC], cps[0:1, :],
                                             AF.Copy, scale=-0.5)
                    nc.vector.tensor_copy(mch2[0:1, :], mch_f[:])
                    nc.vector.tensor_sub(mch_f[:], mch_f[:], mch2[0:1, :].bitcast(F32))
                    nc.vector.tensor_copy(mch_res[:], mch_f[:])
                    # compute engines can't address base_partition=1; DMA can
                    nc.sync.dma_start(mch2[1:2, :], mch_res[:])

            # ---- fused encoder + VQ loop ----
            with tc.tile_pool(name="encw", bufs=1) as encw, \
                 tc.tile_pool(name="encx", bufs=2) as encx, \
                 tc.tile_pool(name="ench", bufs=2) as ench, \
                 tc.tile_pool(name="spp", bufs=2) as spp:
                ew1_t = encw.tile([P, D // P, H], BF16, tag="ew1")
                ew2_t = encw.tile([P, H // P, H], BF16, tag="ew2")
                ew3_t = encw.tile([P, H // P, L], BF16, tag="ew3")
                nc.sync.dma_start(ew1_t[:], ew1_d.ap())
                nc.sync.dma_start(ew2_t[:], ew2_d.ap())
                nc.sync.dma_start(ew3_t[:], ew3_d.ap())

                for st in range(NST):
                    rs = st * RBLK
                    xt = encx.tile([P, D // P, RBLK], BF16, tag="xt")
                    nc.sync.dma_start(xt[:], xTb_r[st])
                    h1 = ench.tile([P, H // P, RBLK], BF16, tag="h")
                    for f in range(H // P):
                        pt = ps_mm.tile([P, RBLK], F32, tag="ps_enc")
                        for d_ in range(D // P):
                            nc.tensor.matmul(
                                pt[:], ew1_t[:, d_, f * P:(f + 1) * P], xt[:, d_, :],
                                start=(d_ == 0), stop=(d_ == D // P - 1))
                        nc.scalar.activation(h1[:, f, :], pt[:], AF.Gelu,
                                             bias=eb1_t[:, f:f + 1])
                    h2 = ench.tile([P, H // P, RBLK], BF16, tag="h")
                    for f in range(H // P):
                        pt = ps_mm.tile([P, RBLK], F32, tag="ps_enc")
                        for d_ in range(H // P):
                            nc.tensor.matmul(
                                pt[:], ew2_t[:, d_, f * P:(f + 1) * P], h1[:, d_, :],
                                start=(d_ == 0), stop=(d_ == H // P - 1))
                        nc.scalar.activation(h2[:, f, :], pt[:], AF.Gelu,
                                             bias=eb2_t[:, f:f + 1])
                    for f in range(L // P):
                        pt = ps_mm.tile([P, RBLK], F32, tag="ps_enc")
                        for d_ in range(H // P):
                            nc.tensor.matmul(
                                pt[:], ew3_t[:, d_, f * P:(f + 1) * P], h2[:, d_, :],
                                start=(d_ == 0), stop=(d_ == H // P - 1))
                        nc.scalar.activation(zeT[:, f, rs:rs + RBLK], pt[:],
                                             AF.Identity, bias=eb3_t[:, f:f + 1])

                    if 2 not in phases:
                        continue
                    # ---- VQ for the two row-tiles of this supertile ----
                    for i in (2 * st, 2 * st + 1):
                        ri = i * P
                        halves = []
                        for hf in range(2):
                            sph = spp.tile([P, KH], F32, tag="sp")
                            for kc in range(NKC // 2):
                                ks = hf * KH + kc * KC
                                dps = ps_big.tile([P, KC], F32, tag="ps_dist")
                                nc.tensor.matmul(dps[:], zeT[:, 0, ri:ri + P],
                                                 et_t[:, 0, ks:ks + KC],
                                                 start=True, stop=False)
                                nc.tensor.matmul(dps[:], zeT[:, 1, ri:ri + P],
                                                 et_t[:, 1, ks:ks + KC],
                                                 start=False, stop=False)
                                nc.tensor.matmul(dps[:], ones2_t[:],
                                                 mch2[:, ks:ks + KC],
                                                 start=False, stop=True)
                                nc.scalar.copy(sph[:, kc * KC:(kc + 1) * KC], dps[:])
                            if p2_level < 2:
                                continue
                            mx = vwork.tile([P, 8], F32, tag="mx8")
                            ix = vwork.tile([P, 8], U32, tag="ix8")
                            nc.vector.max(mx[:], sph[:])
                            nc.vector.max_index(ix[:], mx[:], sph[:])
                            halves.append((mx, ix))
                        if p2_level < 2:
                            continue
                        (mxA, ixA), (mxB, ixB) = halves
                        fA = vwork.tile([P, 1], F32, tag="fA")
                        fB = vwork.tile([P, 1], F32, tag="fB")
                        nc.vector.tensor_copy(fA[:], ixA[:, 0:1])
                        nc.vector.tensor_copy(fB[:], ixB[:, 0:1])
                        nc.vector.tensor_scalar_add(fB[:], fB[:], float(KH))
                        msk = vwork.tile([P, 1], I32, tag="msk")
                        nc.vector.tensor_tensor(msk[:], mxA[:, 0:1], mxB[:, 0:1],
                                                ALU.is_ge)
                        ixf = vwork.tile([P, 1], F32, tag="ixf")
                        nc.vector.select(ixf[:], msk[:], fA[:], fB[:])
                        ixu = vwork.tile([P, 1], U32, tag="ixu")
                        nc.vector.tensor_copy(ixu[:], ixf[:])
                        if p2_level >= 3:
                            # z_discrete: scatter int32 ones at flat offsets
                            off_f = vwork.tile([P, 1], F32, tag="off_f")
                            nc.vector.tensor_scalar(off_f[:], ixf[:], p4096_t[:],
                                                    float(ri * K), ALU.add, ALU.add)
                            off_i = vwork.tile([P, 1], I32, tag="off_i")
                            nc.vector.tensor_copy(off_i[:], off_f[:])
                            nc.gpsimd.indirect_dma_start(
                                out=oh_flat, out_offset=IndirectOffsetOnAxis(
                                    ap=off_i[:], axis=0),
                                in_=ones_i32[:], in_offset=None)
                        if p2_level >= 4:
                            # gather z_q rows from HBM
                            nc.gpsimd.indirect_dma_start(
                                out=zq_all[:, i, :], out_offset=None, in_=ep_d.ap(),
                                in_offset=IndirectOffsetOnAxis(
                                    ap=ixu[:], axis=0))

            et_cm.__exit__(None, None, None)

            # zqT + decoder weights on the right side (into embedT's space)
            zqp = stack.enter_context(tc.tile_pool(name="zqp", bufs=1, side="right"))
            zqT = zqp.tile([P, L // P, NS], F32R, tag="zqT")
            if 3 in phases:
                decw = stack.enter_context(
                    tc.tile_pool(name="decw", bufs=1, side="right"))
                dw2_t = decw.tile([P, H // P, H], F32R, tag="dw2")
                dw3_t = decw.tile([P, H // P, D], F32R, tag="dw3")
                for o in range(H // P):
                    nc.sync.dma_start(dw2_t[:, o], dw2_d.ap()[:, o])
                    nc.sync.dma_start(dw3_t[:, o], dw3_d.ap()[:, o])

            # ---- loop B: z_q transposes + codebook-loss partials ----
            if 2 in phases and p2_level >= 5:
                for i in range(NTILE):
                    ri = i * P
                    for lo in range(L // P):
                        tps = ps_tp.tile([P, P], F32, tag="tp")
                        nc.tensor.transpose(tps[:], zq_all[:, i, lo * P:(lo + 1) * P],
                                            ident_t[:])
                        nc.scalar.copy(zqT[:, lo, ri:ri + P], tps[:])
                    df = vwork.tile([P, L // P, P], F32, tag="df")
                    nc.vector.tensor_sub(df[:], zeT[:, :, ri:ri + P].bitcast(F32),
                                         zqT[:, :, ri:ri + P].bitcast(F32))
                    nc.scalar.activation(df[:], df[:], AF.Square,
                                         accum_out=s2buf[:, i:i + 1])

            vwork_cm.__exit__(None, None, None)
            vq_cm.__exit__(None, None, None)

            # ================= P3: decoder =================
            if 3 in phases:
              with tc.tile_pool(name="decwork", bufs=2) as dwork, \
                 tc.tile_pool(name="decwork1", bufs=1) as dwork1:
                for pr in range(NST // 2):
                    g2s = []
                    for st in (2 * pr, 2 * pr + 1):
                        rs = st * RBLK
                        g1 = dwork.tile([P, H // P, RBLK], F32R, tag="g1")
                        for f in range(H // P):
                            pt = ps_mm.tile([P, RBLK], F32, tag="ps_enc")
                            for d_ in range(L // P):
                                nc.tensor.matmul(
                                    pt[:], dw1_t[:, d_, f * P:(f + 1) * P],
                                    zqT[:, d_, rs:rs + RBLK],
                                    start=(d_ == 0), stop=(d_ == L // P - 1))
                            nc.scalar.activation(g1[:, f, :], pt[:], AF.Gelu,
                                                 bias=db1_t[:, f:f + 1])
                        g2 = dwork.tile([P, H // P, RBLK], F32R, tag="g2")
                        for f in range(H // P):
                            pt = ps_mm.tile([P, RBLK], F32, tag="ps_enc")
                            for d_ in range(H // P):
                                nc.tensor.matmul(
                                    pt[:], dw2_t[:, d_, f * P:(f + 1) * P],
                                    g1[:, d_, :],
                                    start=(d_ == 0), stop=(d_ == H // P - 1))
                            nc.scalar.activation(g2[:, f, :], pt[:], AF.Gelu,
                                                 bias=db2_t[:, f:f + 1])
                        g2s.append((st, g2))
                    for st, g2 in g2s:
                        rs = st * RBLK
                        xp = dwork.tile([P, D // P, RBLK], F32, tag="xp")
                        for f in range(D // P):
                            pt = ps_mm.tile([P, RBLK], F32, tag="ps_enc")
                            for d_ in range(H // P):
                                nc.tensor.matmul(
                                    pt[:], dw3_t[:, d_, f * P:(f + 1) * P],
                                    g2[:, d_, :],
                                    start=(d_ == 0), stop=(d_ == H // P - 1))
                            nc.scalar.activation(xp[:, f, :], pt[:], AF.Sigmoid,
                                                 bias=db3_t[:, f:f + 1])
                        nc.sync.dma_start(xpT_r[st], xp[:])
                        # recon-loss partial: sum((x - x_pred)^2)
                        xtf = dwork1.tile([P, D // P, RBLK], F32, tag="xtf")
                        nc.sync.dma_start(xtf[:], xT_r[st])
                        nc.vector.tensor_sub(xtf[:], xtf[:], xp[:])
                        nc.scalar.activation(xtf[:], xtf[:], AF.Square,
                                             accum_out=s1buf[:, st:st + 1])

            # ================= loss partials out =================
            lp = persist.tile([P, 2], F32, tag="lossp")
            nc.vector.reduce_sum(lp[:, 0:1], s1buf[:], axis=mybir.AxisListType.X)
            nc.vector.reduce_sum(lp[:, 1:2], s2buf[:], axis=mybir.AxisListType.X)
            nc.sync.dma_start(lossp_d.ap(), lp[:])

    nc.finalize()
    return nc


def _wtile(w, dtype=np.float32):
    """[K_in, F] -> [128, K_in//128, F] (partition-major chunks, contiguous
    per partition)."""
    w = np.asarray(w, np.float32)
    out = np.ascontiguousarray(w.reshape(-1, P, w.shape[1]).transpose(1, 0, 2))
    if dtype is not np.float32:
        out = np.ascontiguousarray(out.astype(dtype))
    return out


def _prep_shared(inputs):
    import ml_dtypes
    bf = ml_dtypes.bfloat16
    ep = np.ascontiguousarray(inputs["embed_pool"], dtype=np.float32)
    shared = {
        "eT": _wtile(np.ascontiguousarray(ep.T)),
        "ep": ep,
        "ew1": _wtile(inputs["ew1"], bf),
        "ew2": _wtile(inputs["ew2"], bf),
        "ew3": _wtile(inputs["ew3"], bf),
        "dw1": _wtile(inputs["dw1"]),
        "dw2": _wtile(inputs["dw2"]),
        "dw3": _wtile(inputs["dw3"]),
        "eb1": np.ascontiguousarray(np.asarray(inputs["eb1"], np.float32).reshape(-1, P).T),
        "eb2": np.ascontiguousarray(np.asarray(inputs["eb2"], np.float32).reshape(-1, P).T),
        "eb3": np.ascontiguousarray(np.asarray(inputs["eb3"], np.float32).reshape(-1, P).T),
        "db1": np.ascontiguousarray(np.asarray(inputs["db1"], np.float32).reshape(-1, P).T),
        "db2": np.ascontiguousarray(np.asarray(inputs["db2"], np.float32).reshape(-1, P).T),
        "db3": np.ascontiguousarray(np.asarray(inputs["db3"], np.float32).reshape(-1, P).T),
        "p4096": (np.arange(P, dtype=np.float32) * K).reshape(P, 1),
        "ident": np.eye(P, dtype=np.float32),
        "ones2": np.ones((2, P), np.float32),
        "ones128": np.ones((P, 1), np.float32),
    }
    return shared


def _xtile(x_shard):
    """[NS, D] -> [NST, 128, D//128, RBLK]: xtile[st, p, o, r] =
    x[st*RBLK + r, o*128 + p]."""
    v = x_shard.reshape(NST, RBLK, D // P, P)
    return np.ascontiguousarray(v.transpose(0, 3, 2, 1))


def _xuntile(xt):
    """Inverse of _xtile: [NST, 128, D//128, RBLK] -> [NS, D]."""
    return np.ascontiguousarray(xt.transpose(0, 3, 2, 1).reshape(NS, D))


def _make_in_maps(inputs):
    import ml_dtypes
    x = np.ascontiguousarray(np.asarray(inputs["x"], np.float32))
    shared = _prep_shared(inputs)
    in_maps = []
    for c in range(NCORES):
        m = dict(shared)
        xt = _xtile(x[c * NS:(c + 1) * NS])
        m["xT"] = xt
        m["xTb"] = np.ascontiguousarray(xt.astype(ml_dtypes.bfloat16))
        in_maps.append(m)
    return in_maps


def _run(inputs, trace=False):
    if "nc" not in _cache:
        _cache["nc"] = _build()
    nc = _cache["nc"]
    return _run_nc(nc, inputs, trace)


def _run_nc(nc, inputs, trace=False):
    in_maps = _make_in_maps(inputs)
    res = run_bass_kernel_spmd(nc, in_maps, core_ids=list(range(NCORES)),
                               trace=trace)
    return _assemble(res.results), res


def _assemble(results):
    x_pred = np.empty((N, D), np.float32)
    z_disc = np.empty((N, K), np.int32)
    s1 = 0.0
    s2 = 0.0
    for c, r in enumerate(results):
        x_pred[c * NS:(c + 1) * NS] = _xuntile(r["xpredT"])
        z_disc[c * NS:(c + 1) * NS] = r["onehot"]
        s1 += r["lossp"][:, 0].astype(np.float64).sum()
        s2 += r["lossp"][:, 1].astype(np.float64).sum()
    loss = np.float32((s1 + 1.25 * s2) / N)
    return (x_pred, z_disc, loss)


def kernel(**inputs):
    out, _ = _run(inputs, trace=False)
    return out


def _bench_nc(nc, in_maps, iters):
    """Build the sharded jit once for `nc`, keep inputs device-resident,
    re-donate outputs; return (times, host_outs_of_last_iter)."""
    import time

    import jax
    from jax.sharding import Mesh, NamedSharding, PartitionSpec
    from jax.experimental.shard_map import shard_map

    from concourse import bass2jax as B2J

    B2J.install_neuronx_cc_hook()
    partition_name = nc.partition_id_tensor.name if nc.partition_id_tensor else None
    in_names, out_names, out_avals, zero_outs = [], [], [], []
    for alloc in nc.m.functions[0].allocations:
        if not isinstance(alloc, mybir.MemoryLocationSet):
            continue
        name = alloc.memorylocations[0].name
        if alloc.kind == "ExternalInput":
            if name != partition_name:
                in_names.append(name)
        elif alloc.kind == "ExternalOutput":
            out_names.append(name)
            shape = tuple(alloc.tensor_shape)
            dtype = mybir.dt.np(alloc.dtype)
            out_avals.append(jax.core.ShapedArray(shape, dtype))
            zero_outs.append(np.zeros(shape, dtype))
    n_params = len(in_names)
    n_outs = len(out_avals)
    in_names_all = in_names + out_names + ([partition_name] if partition_name else [])
    donate = tuple(range(n_params, n_params + n_outs))

    def _body(*args):
        operands = list(args)
        if partition_name is not None:
            operands.append(B2J.partition_id_tensor())
        return tuple(B2J._bass_exec_p.bind(
            *operands, out_avals=tuple(out_avals), in_names=tuple(in_names_all),
            out_names=tuple(out_names), lowering_input_output_aliases=(),
            sim_require_finite=True, sim_require_nnan=True, nc=nc))

    devices = jax.devices()[:NCORES]
    mesh = Mesh(np.asarray(devices), ("core",))
    sharded = jax.jit(
        shard_map(_body, mesh=mesh,
                  in_specs=(PartitionSpec("core"),) * (n_params + n_outs),
                  out_specs=(PartitionSpec("core"),) * n_outs, check_rep=False),
        donate_argnums=donate, keep_unused=True)

    sh = NamedSharding(mesh, PartitionSpec("core"))
    concat_in = [
        jax.device_put(
            np.concatenate([np.asarray(in_maps[c][nm]) for c in range(NCORES)], 0), sh)
        for nm in in_names
    ]
    concat_zeros = [
        jax.device_put(np.zeros((NCORES * z.shape[0], *z.shape[1:]), z.dtype), sh)
        for z in zero_outs
    ]
    outs = sharded(*concat_in, *concat_zeros)
    jax.block_until_ready(outs)
    times = []
    for _ in range(iters):
        t0 = time.perf_counter()
        outs = sharded(*concat_in, *outs)
        jax.block_until_ready(outs)
        times.append(time.perf_counter() - t0)
    host_outs = [
        {nm: np.asarray(outs[i]).reshape(NCORES, *out_avals[i].shape)[c]
         for i, nm in enumerate(out_names)}
        for c in range(NCORES)
    ]
    return times, host_outs


def bench(inputs, iters=8, repeat=65):
    """Two-point measurement: time repeat=1 and repeat=R NEFFs in the same
    session; device time = (min(tR) - min(t1)) / (R - 1)."""
    in_maps = _make_in_maps(inputs)
    if "nc" not in _cache:
        _cache["nc"] = _build()
    if ("ncR", repeat) not in _cache:
        _cache[("ncR", repeat)] = _build(repeat=repeat)
    t1, host_outs = _bench_nc(_cache["nc"], in_maps, iters)
    tR, host_outs_R = _bench_nc(_cache[("ncR", repeat)], in_maps, iters)
    dev = (min(tR) - min(t1)) / (repeat - 1)
    return dev, t1, tR, host_outs, host_outs_R
